# revision 21
# baseline (speedup 1.0000x reference)
"""Routed (top-2) MoE via permutation MATMULS — no indirect DMA.

Indirect DMA on trn2 costs ~12us per 128 scattered rows (descriptor-bound,
single SWDGE queue) and same-tensor scatters serialize, so gather/scatter
routing drowns in DMA time.  Instead the token->slot permutation is applied
on the TensorEngine:

  pos[t,e]  exclusive prefix count of selected tokens (triangular matmul)
  pos' = sel ? pos : -1
  P_e[t,s]  = (pos'[t,e] == s)        one-hot [T, C] built by DVE is_equal
  x_e^T     = x^T @ P_e               gather as matmul (exact 0/1 weights)
  w_c       = P_e^T @ w[:,e]          compact per-slot gate weights
  h_e^T     = gelu(W1[e]^T @ x_e^T)   routed mm1 (bf16, C=640 slots)
  y_e       = w_c * (h_e^T)^T @ W2[e] routed mm2, scaled at PSUM eviction
  out      += P_e @ y_e               inverse permutation as matmul, P_e^T
                                      tiles from a replicated-pos is_equal
"""

import sys

if "/opt/trn_rl_repo" not in sys.path:
    sys.path.insert(0, "/opt/trn_rl_repo")

import contextlib

import numpy as np
import ml_dtypes

import concourse.bacc as bacc
import concourse.bass as bass
import concourse.mybir as mybir
import concourse.tile as tile
from concourse.bass import ts
from concourse.bass_utils import run_bass_kernel_spmd
from concourse.masks import make_identity

AF = mybir.ActivationFunctionType
ALU = mybir.AluOpType
AX = mybir.AxisListType
F32 = mybir.dt.float32
BF16 = mybir.dt.bfloat16
I32 = mybir.dt.int32

N_CORES = 8
D = 1024
H = 4096
E = 4
TC = 1024
C = 576                      # per-expert slot capacity (max observed 553)

KD = D // 128                # 8
KH = H // 128                # 32
NT = TC // 512               # 2
CT = TC // 128               # 8
ND_ = D // 512               # 2
MB = 4
MC = (C + 127) // 128        # 5 slot chunks (last one 64 wide)
MCW = [(mc * 128, min(128, C - mc * 128)) for mc in range(MC)]
CSL = [(0, 288), (288, 288)] # even moving-dim slices of C (less MM overhead)

NEG_BIG = -1.0e30
_PROGRAMS = {}
LAST_EXEC_NS = None


def _build_routed_program(has_bg: bool, has_b1: bool, has_b2: bool):
    nc = bacc.Bacc("TRN2", debug=False, num_devices=N_CORES, name="moe_perm")

    xT_d = nc.dram_tensor("xT", [D, TC], F32, kind="ExternalInput")
    xr_d = nc.dram_tensor("xrows", [TC, D], BF16, kind="ExternalInput")
    Wg_d = nc.dram_tensor("Wg", [D, E], F32, kind="ExternalInput")
    W1_d = nc.dram_tensor("W1", [E, D, H], BF16, kind="ExternalInput")
    W2_d = nc.dram_tensor("W2", [E, H, D], BF16, kind="ExternalInput")
    if has_bg:
        bg_d = nc.dram_tensor("bg", [1, E], F32, kind="ExternalInput")
    if has_b1:
        b1_d = nc.dram_tensor("b1c", [128, E, KH], F32, kind="ExternalInput")
    if has_b2:
        b2_d = nc.dram_tensor("b2", [1, E, D], F32, kind="ExternalInput")
    out_d = nc.dram_tensor("out", [TC, D], F32, kind="ExternalOutput")

    with tile.TileContext(nc) as tc:
        with contextlib.ExitStack() as ctx:
            # ---------------- constants ------------------------------------
            const = ctx.enter_context(tc.tile_pool(name="const", bufs=1))
            ident = const.tile([128, 128], F32)
            make_identity(nc, ident[:])
            ones = const.tile([1, 512], F32)
            nc.vector.memset(ones[:], 1.0)
            ones128b = const.tile([128, 128], BF16)
            nc.vector.memset(ones128b[:], 1.0)
            # strict upper triangular (i < j) for exclusive prefix counts
            it_row = const.tile([128, 1], I32)
            nc.gpsimd.iota(it_row[:], pattern=[[1, 1]], base=0, channel_multiplier=1)
            it_col = const.tile([128, 128], I32)
            nc.gpsimd.iota(it_col[:], pattern=[[1, 128]], base=0, channel_multiplier=0)
            triub = const.tile([128, 128], BF16)
            trif = const.tile([128, 128], F32)
            nc.vector.tensor_tensor(
                trif[:], it_row[:].broadcast_to([128, 128]), it_col[:], op=ALU.is_lt
            )
            nc.vector.tensor_copy(triub[:], trif[:])
            # slot-index rows / per-partition slot ids for one-hot builds
            iotaC_i = const.tile([128, C], I32)
            nc.gpsimd.iota(iotaC_i[:], pattern=[[1, C]], base=0, channel_multiplier=0)
            iotaC = const.tile([128, C], F32)
            nc.vector.tensor_copy(iotaC[:], iotaC_i[:])
            siota = const.tile([128, MC], F32)
            rowf = const.tile([128, 1], F32)
            nc.vector.tensor_copy(rowf[:], it_row[:])
            for mc in range(MC):
                nc.vector.tensor_scalar(
                    siota[:, mc : mc + 1], rowf[:], float(mc * 128), None, op0=ALU.add
                )
            # onehot4[:, e*128:(e+1)*128] has row e all-ones (K=4 selector for
            # replicating posT row e across 128 partitions via matmul)
            onehot4 = const.tile([4, E * 128], F32)
            for e in range(E):
                nc.vector.tensor_scalar(
                    onehot4[:, ts(e, 128)], rowf[:4, :].broadcast_to([4, 128]),
                    float(e), None, op0=ALU.is_equal,
                )
            wg_sb = const.tile([128, KD, E], F32)
            nc.sync.dma_start(
                wg_sb[:], Wg_d.ap().rearrange("(k p) e -> p k e", p=128)
            )
            if has_bg:
                bg_sb = const.tile([1, E], F32)
                nc.sync.dma_start(bg_sb[:], bg_d.ap())
            if has_b1:
                b1_sb = const.tile([128, E, KH], F32)
                nc.sync.dma_start(b1_sb[:], b1_d.ap())

            small = ctx.enter_context(tc.tile_pool(name="small", bufs=1))
            gates = small.tile([128, CT, E], F32)
            w_sb = small.tile([128, CT, E], F32)
            posp = small.tile([128, CT, E], F32)    # pos' = sel ? pos : -1
            posT = small.tile([E, TC], F32)
            wT = small.tile([E, TC], F32)

            # x rows (bf16) resident: lhsT tiles for the gather matmul
            xs_p = ctx.enter_context(tc.tile_pool(name="xsb", bufs=1))
            x_sb = xs_p.tile([128, CT, D], BF16)
            nc.sync.dma_start(
                x_sb[:], xr_d.ap().rearrange("(c p) d -> p c d", p=128)
            )
            out_p = ctx.enter_context(tc.tile_pool(name="outsb", bufs=1))
            out_sb = out_p.tile([128, CT, D], F32)

            # ---------------- gates (fp32) ---------------------------------
            with contextlib.ExitStack() as gctx:
                x32_p = gctx.enter_context(tc.tile_pool(name="x32", bufs=3))
                pg = gctx.enter_context(tc.tile_pool(name="pg", bufs=2, space="PSUM"))
                ptrp = gctx.enter_context(tc.tile_pool(name="ptrp", bufs=2, space="PSUM"))
                gT_p = gctx.enter_context(tc.tile_pool(name="gT", bufs=1))
                gT = gT_p.tile([E, TC], F32)

                g_ps = [pg.tile([E, 512], F32, name=f"g_ps{n}") for n in range(NT)]
                for k in range(KD):
                    x32 = x32_p.tile([128, TC], F32, name="x32")
                    nc.sync.dma_start(x32[:], xT_d.ap()[ts(k, 128), :])
                    for n in range(NT):
                        nc.tensor.matmul(
                            g_ps[n][:],
                            lhsT=wg_sb[:, k, :],
                            rhs=x32[:, ts(n, 512)],
                            start=(k == 0),
                            stop=(not has_bg) and k == KD - 1,
                        )
                if has_bg:
                    for n in range(NT):
                        nc.tensor.matmul(
                            g_ps[n][:], lhsT=bg_sb[:1, :], rhs=ones[:1, :],
                            start=False, stop=True,
                        )
                for n in range(NT):
                    nc.vector.tensor_copy(gT[:, ts(n, 512)], g_ps[n][:])
                for c in range(CT):
                    g_tr = ptrp.tile([128, E], F32, name="g_tr")
                    nc.tensor.transpose(g_tr[:], gT[:, ts(c, 128)], ident[:E, :E])
                    nc.vector.tensor_copy(gates[:, c, :], g_tr[:])

            # ---------------- top-2 + softmax ------------------------------
            tk = ctx.enter_context(tc.tile_pool(name="topk", bufs=1))
            m1 = tk.tile([128, CT, 1], F32)
            m2 = tk.tile([128, CT, 1], F32)
            maskA = tk.tile([128, CT, E], F32)
            maskB = tk.tile([128, CT, E], F32)
            g2 = tk.tile([128, CT, E], F32)
            expd = tk.tile([128, CT, 1], F32)
            s1 = tk.tile([128, CT, 1], F32)
            s2 = tk.tile([128, CT, 1], F32)
            selb = tk.tile([128, CT, E], BF16)

            nc.vector.reduce_max(m1[:], gates[:], axis=AX.X)
            nc.vector.tensor_tensor(
                maskA[:], gates[:], m1[:].broadcast_to([128, CT, E]), op=ALU.is_equal
            )
            nc.vector.tensor_scalar(g2[:], maskA[:], NEG_BIG_ := -1.0e30, None, op0=ALU.mult)
            nc.vector.tensor_add(g2[:], g2[:], gates[:])
            nc.vector.reduce_max(m2[:], g2[:], axis=AX.X)
            nc.vector.tensor_tensor(
                maskB[:], g2[:], m2[:].broadcast_to([128, CT, E]), op=ALU.is_equal
            )
            nc.vector.tensor_sub(expd[:], m2[:], m1[:])
            nc.scalar.activation(expd[:], expd[:], AF.Exp)
            nc.vector.tensor_scalar(s1[:], expd[:], 1.0, None, op0=ALU.add)
            nc.vector.reciprocal(s1[:], s1[:])
            nc.vector.tensor_mul(s2[:], expd[:], s1[:])
            nc.vector.tensor_tensor(
                w_sb[:], maskA[:], s1[:].broadcast_to([128, CT, E]), op=ALU.mult
            )
            nc.vector.tensor_tensor(
                g2[:], maskB[:], s2[:].broadcast_to([128, CT, E]), op=ALU.mult
            )
            nc.vector.tensor_add(w_sb[:], w_sb[:], g2[:])
            # sel in g2
            nc.vector.tensor_add(g2[:], maskA[:], maskB[:])
            nc.vector.tensor_copy(selb[:], g2[:])

            # ---------------- prefix positions + pos' ----------------------
            with tc.tile_pool(name="ppre", bufs=4, space="PSUM") as ppre:
                for mc in range(CT):
                    pps = ppre.tile([128, E], F32, name="pps")
                    for kc in range(mc + 1):
                        nc.tensor.matmul(
                            pps[:],
                            lhsT=ones128b[:] if kc < mc else triub[:],
                            rhs=selb[:, kc, :],
                            start=(kc == 0),
                            stop=(kc == mc),
                        )
                    nc.vector.tensor_copy(posp[:, mc, :], pps[:])
            # pos' = pos*sel + sel - 1  (-1 for unselected tokens)
            nc.vector.tensor_mul(posp[:], posp[:], g2[:])
            nc.vector.tensor_add(posp[:], posp[:], g2[:])
            nc.vector.tensor_scalar(posp[:], posp[:], -1.0, None, op0=ALU.add)

            # posT[e, t] via PE transpose (for replicated-pos P^T build)
            with tc.tile_pool(name="ptp", bufs=2, space="PSUM") as ptp:
                for c in range(CT):
                    tpp = ptp.tile([E, 128], F32, name="tpp")
                    nc.tensor.transpose(tpp[:], posp[:, c, :], ident[:])
                    nc.vector.tensor_copy(posT[:, ts(c, 128)], tpp[:])
                for c in range(CT):
                    twp = ptp.tile([E, 128], F32, name="twp", tag="tpp")
                    nc.tensor.transpose(twp[:], w_sb[:, c, :], ident[:])
                    nc.vector.tensor_copy(wT[:, ts(c, 128)], twp[:])

            if has_b2:
                b2r_p = ctx.enter_context(tc.tile_pool(name="b2r", bufs=1))
                b2row_p = ctx.enter_context(tc.tile_pool(name="b2row", bufs=1))
                b2rep = b2r_p.tile([128, E, D], BF16)
                b2row = b2row_p.tile([1, E, D], F32)
                nc.sync.dma_start(b2row[:], b2_d.ap())
                with tc.tile_pool(name="pb2", bufs=2, space="PSUM") as pb2:
                    for e in range(E):
                        for nd in range(ND_):
                            b2ps = pb2.tile([128, 512], F32, name="b2ps")
                            nc.tensor.matmul(
                                b2ps[:], lhsT=ones[:1, :128],
                                rhs=b2row[:, e, ts(nd, 512)],
                                start=True, stop=True,
                            )
                            nc.vector.tensor_copy(b2rep[:, e, ts(nd, 512)], b2ps[:])

            # ---------------- expert loop -----------------------------------
            pe_p = ctx.enter_context(tc.tile_pool(name="pe", bufs=2))      # P_e / P_e^T
            xe_p = ctx.enter_context(tc.tile_pool(name="xe", bufs=2))      # xT_e / y_e
            hT_p = ctx.enter_context(tc.tile_pool(name="hT", bufs=1))
            pr_p = ctx.enter_context(tc.tile_pool(name="posrep", bufs=1))
            w1_p = ctx.enter_context(tc.tile_pool(name="w1", bufs=10))
            w2_p = ctx.enter_context(tc.tile_pool(name="w2", bufs=36))
            pA = ctx.enter_context(tc.tile_pool(name="pA", bufs=3, space="PSUM"))
            pB = ctx.enter_context(tc.tile_pool(name="pB", bufs=3, space="PSUM"))
            pM = ctx.enter_context(tc.tile_pool(name="pM", bufs=2, space="PSUM"))

            for e in range(E):
                # ---- P_e one-hot [t, s] and compact gate weights ----
                Pe = pe_p.tile([128, CT, C], BF16, name="Pe", tag="pe")
                for c in range(CT):
                    nc.vector.tensor_scalar(
                        Pe[:, c, :], iotaC[:], posp[:, c, e : e + 1],
                        None, op0=ALU.is_equal,
                    )
                # ---- gather-as-matmul: xT_e[d, s] = x^T @ P_e ----
                xe = xe_p.tile([128, KD, C], BF16, name="xe", tag="xe")
                for dk in range(KD):
                    for n0, nw in CSL:
                        gps = pA.tile([128, 512], F32, name="gps", tag="pa")
                        for kc in range(CT):
                            nc.tensor.matmul(
                                gps[:, :nw],
                                lhsT=x_sb[:, kc, ts(dk, 128)],
                                rhs=Pe[:, kc, n0 : n0 + nw],
                                start=(kc == 0),
                                stop=(kc == CT - 1),
                            )
                        nc.vector.tensor_copy(xe[:, dk, n0 : n0 + nw], gps[:, :nw])

                # ---- mm1 ----
                hT = hT_p.tile([128, KH, C], BF16, name="hT")
                for mb in range(MB):
                    w1t = []
                    for k in range(KD):
                        t = w1_p.tile([128, 1024], BF16, name="w1t")
                        nc.sync.dma_start(
                            t[:], W1_d.ap()[e, ts(k, 128), ts(mb, 1024)]
                        )
                        w1t.append(t)
                    for mi in range(8):
                        m = mb * 8 + mi
                        for n0, nw in CSL:
                            ps = pA.tile([128, 512], F32, name="ps1", tag="pa")
                            for k in range(KD):
                                nc.tensor.matmul(
                                    ps[:, :nw],
                                    lhsT=w1t[k][:, ts(mi, 128)],
                                    rhs=xe[:, k, n0 : n0 + nw],
                                    start=(k == 0),
                                    stop=(k == KD - 1),
                                )
                            bias = b1_sb[:, e, m : m + 1] if has_b1 else 0.0
                            nc.scalar.activation(
                                hT[:, m, n0 : n0 + nw], ps[:, :nw], AF.Gelu,
                                bias=bias,
                            )

                # ---- mm2 (scaled at eviction) -> y_e bf16 ----
                ye = xe_p.tile([128, MC, D], BF16, name="ye", tag="xe")
                for mc, (s0, sw) in enumerate(MCW):
                    if sw < 128:
                        # pad dead slot rows so the combine matmul sees zeros
                        nc.vector.memset(ye[sw:, mc, :], 0.0)
                for nd in range(ND_):
                    w2t = []
                    for k in range(KH):
                        t = w2_p.tile([128, 512], BF16, name="w2t")
                        nc.sync.dma_start(
                            t[:], W2_d.ap()[e, ts(k, 128), ts(nd, 512)]
                        )
                        w2t.append(t)
                    for mc, (s0, sw) in enumerate(MCW):
                        ps = pB.tile([128, 512], F32, name="ps2", tag="pb")
                        for k in range(KH):
                            nc.tensor.matmul(
                                ps[:sw, :],
                                lhsT=hT[:, k, s0 : s0 + sw],
                                rhs=w2t[k][:],
                                start=(k == 0),
                                stop=(k == KH - 1),
                            )
                        nc.vector.tensor_copy(
                            ye[:sw, mc, ts(nd, 512)], ps[:sw, :]
                        )

                # ---- inverse permutation: out += P_e @ y_e ----
                # P_e^T tiles from replicated pos' (exact integer compare)
                prep = pr_p.tile([128, TC], F32, name="prep")
                wrep = pr_p.tile([128, TC], F32, name="wrep")
                for n in range(NT):
                    pps = pM.tile([128, 512], F32, name="prps", tag="pm")
                    nc.tensor.matmul(
                        pps[:], lhsT=onehot4[:, ts(e, 128)],
                        rhs=posT[:, ts(n, 512)],
                        start=True, stop=True,
                    )
                    nc.vector.tensor_copy(prep[:, ts(n, 512)], pps[:])
                    wps2 = pM.tile([128, 512], F32, name="wps2", tag="pm")
                    nc.tensor.matmul(
                        wps2[:], lhsT=onehot4[:, ts(e, 128)],
                        rhs=wT[:, ts(n, 512)],
                        start=True, stop=True,
                    )
                    nc.vector.tensor_copy(wrep[:, ts(n, 512)], wps2[:])
                PeT = pe_p.tile([128, MC, TC], BF16, name="PeT", tag="pe")
                for mc in range(MC):
                    nc.vector.tensor_scalar(
                        PeT[:, mc, :], prep[:], siota[:, mc : mc + 1],
                        None, op0=ALU.is_equal,
                    )
                    nc.vector.tensor_tensor(
                        PeT[:, mc, :], PeT[:, mc, :], wrep[:], op=ALU.mult
                    )
                for c in range(CT):
                    for nd in range(ND_):
                        ps = pB.tile([128, 512], F32, name="pso", tag="pb")
                        for mc in range(MC):
                            nc.tensor.matmul(
                                ps[:],
                                lhsT=PeT[:, mc, ts(c, 128)],
                                rhs=ye[:, mc, ts(nd, 512)],
                                start=(mc == 0),
                                stop=(mc == MC - 1),
                            )
                        dst = out_sb[:, c, ts(nd, 512)]
                        if e == 0:
                            nc.vector.tensor_copy(dst, ps[:])
                        else:
                            nc.vector.tensor_add(dst, dst, ps[:])
                        if e == E - 1 and not has_b2:
                            nc.sync.dma_start(
                                out_d.ap()[ts(c, 128), ts(nd, 512)], dst
                            )

            # ---- optional b2 term ----
            if has_b2:
                tmp_p = ctx.enter_context(tc.tile_pool(name="tmpb2", bufs=3))
                for c in range(CT):
                    for nd in range(ND_):
                        dst = out_sb[:, c, ts(nd, 512)]
                        for e in range(E):
                            tmp = tmp_p.tile([128, 512], F32, name="b2t")
                            nc.vector.tensor_scalar(
                                tmp[:], b2rep[:, e, ts(nd, 512)],
                                w_sb[:, c, e : e + 1], None, op0=ALU.mult,
                            )
                            nc.vector.tensor_add(dst, dst, tmp[:])

            if has_b2:
                for c in range(CT):
                    nc.sync.dma_start(out_d.ap()[ts(c, 128), :], out_sb[:, c, :])

    nc.compile()
    return nc


def _build_dense_program(has_bg: bool, has_b1: bool, has_b2: bool):
    nc = bacc.Bacc("TRN2", debug=False, num_devices=N_CORES, name="moe_dense")

    xT_d = nc.dram_tensor("xT", [D, TC], F32, kind="ExternalInput")
    Wg_d = nc.dram_tensor("Wg", [D, E], F32, kind="ExternalInput")
    W1_d = nc.dram_tensor("W1", [E, D, H], BF16, kind="ExternalInput")
    W2_d = nc.dram_tensor("W2", [E, H, D], BF16, kind="ExternalInput")
    if has_bg:
        bg_d = nc.dram_tensor("bg", [1, E], F32, kind="ExternalInput")
    if has_b1:
        b1_d = nc.dram_tensor("b1c", [128, E, H // 128], F32, kind="ExternalInput")
    if has_b2:
        b2_d = nc.dram_tensor("b2", [1, E, D], F32, kind="ExternalInput")
    out_d = nc.dram_tensor("out", [TC, D], F32, kind="ExternalOutput")

    KD = D // 128        # 8   k-chunks of D
    KH = H // 128        # 32  k-chunks of H
    NT = TC // 512       # 2   moving-dim slices of tokens
    CT = TC // 128       # 8   token chunks (psum partition tiles)
    ND_ = D // 512       # 2   output D slices
    MB = 4               # mm1 H-chunk blocks (KH/8 per block)

    with tile.TileContext(nc) as tc:
        import contextlib

        with contextlib.ExitStack() as ctx:
            # ---------------- constants / small persistent tiles ------------
            const = ctx.enter_context(tc.tile_pool(name="const", bufs=1))
            ident = const.tile([128, 128], F32)
            make_identity(nc, ident[:])
            ones = const.tile([1, 512], F32)
            nc.vector.memset(ones[:], 1.0)
            wg_sb = const.tile([128, KD, E], F32)
            nc.sync.dma_start(
                wg_sb[:], Wg_d.ap().rearrange("(k p) e -> p k e", p=128)
            )
            if has_bg:
                bg_sb = const.tile([1, E], F32)
                nc.sync.dma_start(bg_sb[:], bg_d.ap())
            if has_b1:
                b1_sb = const.tile([128, E, KH], F32)
                nc.sync.dma_start(b1_sb[:], b1_d.ap())

            small = ctx.enter_context(tc.tile_pool(name="small", bufs=1))
            gates = small.tile([128, CT, E], F32)
            w_sb = small.tile([128, CT, E], F32)

            # persistent big tensors
            xbf_p = ctx.enter_context(tc.tile_pool(name="xbf", bufs=1))
            xbf = xbf_p.tile([128, KD, TC], BF16)
            out_p = ctx.enter_context(tc.tile_pool(name="outsb", bufs=1))
            out_sb = out_p.tile([128, CT, D], F32)
            hT_p = ctx.enter_context(tc.tile_pool(name="hT", bufs=1))

            # ---------------- gates (fp32) + x cast --------------------------
            with contextlib.ExitStack() as gctx:
                x32_p = gctx.enter_context(tc.tile_pool(name="x32", bufs=3))
                pg = gctx.enter_context(
                    tc.tile_pool(name="pg", bufs=2, space="PSUM")
                )
                ptr = gctx.enter_context(
                    tc.tile_pool(name="ptr", bufs=2, space="PSUM")
                )
                gT_p = gctx.enter_context(tc.tile_pool(name="gT", bufs=1))
                gT = gT_p.tile([E, TC], F32)

                g_ps = [pg.tile([E, 512], F32, name=f"g_ps{n}") for n in range(NT)]
                for k in range(KD):
                    x32 = x32_p.tile([128, TC], F32, name="x32")
                    nc.sync.dma_start(x32[:], xT_d.ap()[ts(k, 128), :])
                    nc.vector.tensor_copy(xbf[:, k, :], x32[:])
                    for n in range(NT):
                        nc.tensor.matmul(
                            g_ps[n][:],
                            lhsT=wg_sb[:, k, :],
                            rhs=x32[:, ts(n, 512)],
                            start=(k == 0),
                            stop=(not has_bg) and k == KD - 1,
                        )
                if has_bg:
                    for n in range(NT):
                        nc.tensor.matmul(
                            g_ps[n][:],
                            lhsT=bg_sb[:1, :],
                            rhs=ones[:1, :],
                            start=False,
                            stop=True,
                        )
                for n in range(NT):
                    nc.vector.tensor_copy(gT[:, ts(n, 512)], g_ps[n][:])

                # transpose [E, T] -> [T-chunk, E] tiles
                for c in range(CT):
                    g_tr = ptr.tile([128, E], F32, name="g_tr")
                    nc.tensor.transpose(
                        g_tr[:], gT[:, ts(c, 128)], ident[:E, :E]
                    )
                    nc.vector.tensor_copy(gates[:, c, :], g_tr[:])

            # ---------------- top-2 + softmax -> w ---------------------------
            tk = ctx.enter_context(tc.tile_pool(name="topk", bufs=1))
            m1 = tk.tile([128, CT, 1], F32)
            m2 = tk.tile([128, CT, 1], F32)
            mask1 = tk.tile([128, CT, E], F32)
            mask2 = tk.tile([128, CT, E], F32)
            g2 = tk.tile([128, CT, E], F32)
            expd = tk.tile([128, CT, 1], F32)
            s1 = tk.tile([128, CT, 1], F32)
            s2 = tk.tile([128, CT, 1], F32)

            nc.vector.reduce_max(m1[:], gates[:], axis=AX.X)
            nc.vector.tensor_tensor(
                mask1[:], gates[:], m1[:].broadcast_to([128, CT, E]), op=ALU.is_equal
            )
            nc.vector.tensor_scalar(g2[:], mask1[:], NEG_BIG, None, op0=ALU.mult)
            nc.vector.tensor_add(g2[:], g2[:], gates[:])
            nc.vector.reduce_max(m2[:], g2[:], axis=AX.X)
            nc.vector.tensor_tensor(
                mask2[:], g2[:], m2[:].broadcast_to([128, CT, E]), op=ALU.is_equal
            )
            nc.vector.tensor_sub(expd[:], m2[:], m1[:])
            nc.scalar.activation(expd[:], expd[:], AF.Exp)
            nc.vector.tensor_scalar(s1[:], expd[:], 1.0, None, op0=ALU.add)
            nc.vector.reciprocal(s1[:], s1[:])
            nc.vector.tensor_mul(s2[:], expd[:], s1[:])
            nc.vector.tensor_tensor(
                mask1[:], mask1[:], s1[:].broadcast_to([128, CT, E]), op=ALU.mult
            )
            nc.vector.tensor_tensor(
                mask2[:], mask2[:], s2[:].broadcast_to([128, CT, E]), op=ALU.mult
            )
            nc.vector.tensor_add(w_sb[:], mask1[:], mask2[:])

            # optional b2 row replicated across partitions (via ones outer-prod)
            if has_b2:
                b2r_p = ctx.enter_context(tc.tile_pool(name="b2r", bufs=1))
                b2row_p = ctx.enter_context(tc.tile_pool(name="b2row", bufs=1))
                b2rep = b2r_p.tile([128, E, D], BF16)
                b2row = b2row_p.tile([1, E, D], F32)
                nc.sync.dma_start(b2row[:], b2_d.ap())
                with tc.tile_pool(name="pb2", bufs=2, space="PSUM") as pb2:
                    for e in range(E):
                        for nd in range(ND_):
                            b2ps = pb2.tile([128, 512], F32, name="b2ps")
                            nc.tensor.matmul(
                                b2ps[:],
                                lhsT=ones[:1, :128],
                                rhs=b2row[:, e, ts(nd, 512)],
                                start=True,
                                stop=True,
                            )
                            nc.vector.tensor_copy(b2rep[:, e, ts(nd, 512)], b2ps[:])

            # ---------------- expert loop ------------------------------------
            w1_p = ctx.enter_context(tc.tile_pool(name="w1", bufs=10))
            w2_p = ctx.enter_context(tc.tile_pool(name="w2", bufs=34))
            pmm1 = ctx.enter_context(tc.tile_pool(name="pmm1", bufs=4, space="PSUM"))
            pmm2 = ctx.enter_context(tc.tile_pool(name="pmm2", bufs=3, space="PSUM"))
            tmp_p = ctx.enter_context(tc.tile_pool(name="tmp", bufs=3))

            for e in range(E):
                hT = hT_p.tile([128, KH, TC], BF16, name="hT")

                # ---- mm1: hT = gelu(W1[e].T @ xT + b1[e]) ----
                for mb in range(MB):
                    w1t = []
                    for k in range(KD):
                        t = w1_p.tile([128, 1024], BF16, name="w1t")
                        nc.sync.dma_start(
                            t[:], W1_d.ap()[e, ts(k, 128), ts(mb, 1024)]
                        )
                        w1t.append(t)
                    for mi in range(8):
                        m = mb * 8 + mi
                        for n in range(NT):
                            ps = pmm1.tile([128, 512], F32, name="ps1")
                            for k in range(KD):
                                nc.tensor.matmul(
                                    ps[:],
                                    lhsT=w1t[k][:, ts(mi, 128)],
                                    rhs=xbf[:, k, ts(n, 512)],
                                    start=(k == 0),
                                    stop=(k == KD - 1),
                                )
                            bias = b1_sb[:, e, m : m + 1] if has_b1 else 0.0
                            nc.scalar.activation(
                                hT[:, m, ts(n, 512)], ps[:], AF.Gelu, bias=bias
                            )

                # ---- mm2: out += w[:, e] * (hT.T @ W2[e]) ----
                for nd in range(ND_):
                    w2t = []
                    for k in range(KH):
                        t = w2_p.tile([128, 512], BF16, name="w2t")
                        nc.sync.dma_start(
                            t[:], W2_d.ap()[e, ts(k, 128), ts(nd, 512)]
                        )
                        w2t.append(t)
                    for c in range(CT):
                        ps = pmm2.tile([128, 512], F32, name="ps2")
                        for k in range(KH):
                            nc.tensor.matmul(
                                ps[:],
                                lhsT=hT[:, k, ts(c, 128)],
                                rhs=w2t[k][:],
                                start=(k == 0),
                                stop=(k == KH - 1),
                            )
                        wv = w_sb[:, c, e : e + 1]
                        dst = out_sb[:, c, ts(nd, 512)]
                        if e == 0:
                            nc.vector.tensor_scalar(dst, ps[:], wv, None, op0=ALU.mult)
                        else:
                            tmp = tmp_p.tile([128, 512], F32, name="acc_tmp")
                            nc.vector.tensor_scalar(
                                tmp[:], ps[:], wv, None, op0=ALU.mult
                            )
                            nc.vector.tensor_add(dst, dst, tmp[:])

            # ---- optional b2 term: out += w @ b2 ----
            if has_b2:
                for c in range(CT):
                    for nd in range(ND_):
                        dst = out_sb[:, c, ts(nd, 512)]
                        for e in range(E):
                            tmp = tmp_p.tile([128, 512], F32, name="acc_tmp")
                            nc.vector.tensor_scalar(
                                tmp[:],
                                b2rep[:, e, ts(nd, 512)],
                                w_sb[:, c, e : e + 1],
                                None,
                                op0=ALU.mult,
                            )
                            nc.vector.tensor_add(dst, dst, tmp[:])

            # ---- store ----
            for c in range(CT):
                nc.sync.dma_start(out_d.ap()[ts(c, 128), :], out_sb[:, c, :])

    nc.compile()
    return nc


def _capacity_ok(xf, Wg, bg):
    """Host-side check only: if any (core, expert) routed count could exceed
    the compiled capacity C, fall back to the dense program (all math still
    happens on device either way)."""
    g = xf @ Wg + bg
    top2 = np.argpartition(-g, 2, axis=1)[:, :2]
    for c in range(N_CORES):
        t = top2[c * TC : (c + 1) * TC]
        for e in range(E):
            if (t == e).sum() > C - 8:
                return False
    return True


def kernel(x, Wg, bg, W1, b1, W2, b2):
    global LAST_EXEC_NS
    import os

    x = np.asarray(x, dtype=np.float32)
    Wg = np.asarray(Wg, dtype=np.float32)
    bg = np.asarray(bg, dtype=np.float32)
    W1 = np.asarray(W1, dtype=np.float32)
    b1 = np.asarray(b1, dtype=np.float32)
    W2 = np.asarray(W2, dtype=np.float32)
    b2 = np.asarray(b2, dtype=np.float32)

    B, S, _ = x.shape
    xf = x.reshape(-1, D)
    assert xf.shape[0] == N_CORES * TC

    has_bg = bool(np.any(bg))
    has_b1 = bool(np.any(b1))
    has_b2 = bool(np.any(b2))
    routed = _capacity_ok(xf, Wg, bg)
    key = (routed, has_bg, has_b1, has_b2)
    if key not in _PROGRAMS:
        build = _build_routed_program if routed else _build_dense_program
        _PROGRAMS[key] = build(has_bg, has_b1, has_b2)
    nc = _PROGRAMS[key]

    W1bf = W1.astype(ml_dtypes.bfloat16)
    W2bf = W2.astype(ml_dtypes.bfloat16)

    in_maps = []
    for c in range(N_CORES):
        shard = xf[c * TC : (c + 1) * TC]
        m = {
            "xT": np.ascontiguousarray(shard.T),
            "Wg": Wg,
            "W1": W1bf,
            "W2": W2bf,
        }
        if routed:
            m["xrows"] = shard.astype(ml_dtypes.bfloat16)
        if has_bg:
            m["bg"] = bg.reshape(1, E)
        if has_b1:
            m["b1c"] = np.ascontiguousarray(
                b1.reshape(E, H // 128, 128).transpose(2, 0, 1)
            )
        if has_b2:
            m["b2"] = b2.reshape(1, E, D)
        in_maps.append(m)

    trace = bool(os.environ.get("KERNEL_TRACE"))
    res = run_bass_kernel_spmd(
        nc, in_maps, core_ids=list(range(N_CORES)), trace=trace
    )
    LAST_EXEC_NS = res.exec_time_ns

    out = np.concatenate([res.results[c]["out"] for c in range(N_CORES)], axis=0)
    return out.reshape(B, S, D)


# revision 22
# speedup vs baseline: 1.0023x; 1.0023x over previous
"""Routed (top-2) MoE via permutation MATMULS — no indirect DMA.

Indirect DMA on trn2 costs ~12us per 128 scattered rows (descriptor-bound,
single SWDGE queue) and same-tensor scatters serialize, so gather/scatter
routing drowns in DMA time.  Instead the token->slot permutation is applied
on the TensorEngine:

  pos[t,e]  exclusive prefix count of selected tokens (triangular matmul)
  pos' = sel ? pos : -1
  P_e[t,s]  = (pos'[t,e] == s)        one-hot [T, C] built by DVE is_equal
  x_e^T     = x^T @ P_e               gather as matmul (exact 0/1 weights)
  w_c       = P_e^T @ w[:,e]          compact per-slot gate weights
  h_e^T     = gelu(W1[e]^T @ x_e^T)   routed mm1 (bf16, C=640 slots)
  y_e       = w_c * (h_e^T)^T @ W2[e] routed mm2, scaled at PSUM eviction
  out      += P_e @ y_e               inverse permutation as matmul, P_e^T
                                      tiles from a replicated-pos is_equal
"""

import sys

if "/opt/trn_rl_repo" not in sys.path:
    sys.path.insert(0, "/opt/trn_rl_repo")

import contextlib

import numpy as np
import ml_dtypes

import concourse.bacc as bacc
import concourse.bass as bass
import concourse.mybir as mybir
import concourse.tile as tile
from concourse.bass import ts
from concourse.bass_utils import run_bass_kernel_spmd
from concourse.masks import make_identity

AF = mybir.ActivationFunctionType
ALU = mybir.AluOpType
AX = mybir.AxisListType
F32 = mybir.dt.float32
BF16 = mybir.dt.bfloat16
I32 = mybir.dt.int32

N_CORES = 8
D = 1024
H = 4096
E = 4
TC = 1024
C = 576                      # per-expert slot capacity (max observed 553)

KD = D // 128                # 8
KH = H // 128                # 32
NT = TC // 512               # 2
CT = TC // 128               # 8
ND_ = D // 512               # 2
MB = 4
MC = (C + 127) // 128        # 5 slot chunks (last one 64 wide)
MCW = [(mc * 128, min(128, C - mc * 128)) for mc in range(MC)]
CSL = [(0, 288), (288, 288)] # even moving-dim slices of C (less MM overhead)

NEG_BIG = -1.0e30
_PROGRAMS = {}
LAST_EXEC_NS = None


def _build_routed_program(has_bg: bool, has_b1: bool, has_b2: bool):
    nc = bacc.Bacc("TRN2", debug=False, num_devices=N_CORES, name="moe_perm")

    xT_d = nc.dram_tensor("xT", [D, TC], F32, kind="ExternalInput")
    xr_d = nc.dram_tensor("xrows", [TC, D], BF16, kind="ExternalInput")
    Wg_d = nc.dram_tensor("Wg", [D, E], F32, kind="ExternalInput")
    W1_d = nc.dram_tensor("W1", [E, D, H], BF16, kind="ExternalInput")
    W2_d = nc.dram_tensor("W2", [E, H, D], BF16, kind="ExternalInput")
    if has_bg:
        bg_d = nc.dram_tensor("bg", [1, E], F32, kind="ExternalInput")
    if has_b1:
        b1_d = nc.dram_tensor("b1c", [128, E, KH], F32, kind="ExternalInput")
    if has_b2:
        b2_d = nc.dram_tensor("b2", [1, E, D], F32, kind="ExternalInput")
    out_d = nc.dram_tensor("out", [TC, D], F32, kind="ExternalOutput")

    with tile.TileContext(nc) as tc:
        with contextlib.ExitStack() as ctx:
            # ---------------- constants ------------------------------------
            const = ctx.enter_context(tc.tile_pool(name="const", bufs=1))
            ident = const.tile([128, 128], F32)
            make_identity(nc, ident[:])
            ones = const.tile([1, 512], F32)
            nc.vector.memset(ones[:], 1.0)
            ones128b = const.tile([128, 128], BF16)
            nc.vector.memset(ones128b[:], 1.0)
            # strict upper triangular (i < j) for exclusive prefix counts
            it_row = const.tile([128, 1], I32)
            nc.gpsimd.iota(it_row[:], pattern=[[1, 1]], base=0, channel_multiplier=1)
            it_col = const.tile([128, 128], I32)
            nc.gpsimd.iota(it_col[:], pattern=[[1, 128]], base=0, channel_multiplier=0)
            triub = const.tile([128, 128], BF16)
            trif = const.tile([128, 128], F32)
            nc.vector.tensor_tensor(
                trif[:], it_row[:].broadcast_to([128, 128]), it_col[:], op=ALU.is_lt
            )
            nc.vector.tensor_copy(triub[:], trif[:])
            # slot-index rows / per-partition slot ids for one-hot builds
            iotaC_i = const.tile([128, C], I32)
            nc.gpsimd.iota(iotaC_i[:], pattern=[[1, C]], base=0, channel_multiplier=0)
            iotaC = const.tile([128, C], F32)
            nc.vector.tensor_copy(iotaC[:], iotaC_i[:])
            siota = const.tile([128, MC], F32)
            rowf = const.tile([128, 1], F32)
            nc.vector.tensor_copy(rowf[:], it_row[:])
            for mc in range(MC):
                nc.vector.tensor_scalar(
                    siota[:, mc : mc + 1], rowf[:], float(mc * 128), None, op0=ALU.add
                )
            # onehot4[:, e*128:(e+1)*128] has row e all-ones (K=4 selector for
            # replicating posT row e across 128 partitions via matmul)
            onehot4 = const.tile([4, E * 128], F32)
            for e in range(E):
                nc.vector.tensor_scalar(
                    onehot4[:, ts(e, 128)], rowf[:4, :].broadcast_to([4, 128]),
                    float(e), None, op0=ALU.is_equal,
                )
            wg_sb = const.tile([128, KD, E], F32)
            nc.sync.dma_start(
                wg_sb[:], Wg_d.ap().rearrange("(k p) e -> p k e", p=128)
            )
            if has_bg:
                bg_sb = const.tile([1, E], F32)
                nc.sync.dma_start(bg_sb[:], bg_d.ap())
            if has_b1:
                b1_sb = const.tile([128, E, KH], F32)
                nc.sync.dma_start(b1_sb[:], b1_d.ap())

            small = ctx.enter_context(tc.tile_pool(name="small", bufs=1))
            gates = small.tile([128, CT, E], F32)
            w_sb = small.tile([128, CT, E], F32)
            posp = small.tile([128, CT, E], F32)    # pos' = sel ? pos : -1
            posT = small.tile([E, TC], F32)
            wT = small.tile([E, TC], F32)

            # x rows (bf16) resident: lhsT tiles for the gather matmul
            xs_p = ctx.enter_context(tc.tile_pool(name="xsb", bufs=1))
            x_sb = xs_p.tile([128, CT, D], BF16)
            nc.sync.dma_start(
                x_sb[:], xr_d.ap().rearrange("(c p) d -> p c d", p=128)
            )
            out_p = ctx.enter_context(tc.tile_pool(name="outsb", bufs=1))
            out_sb = out_p.tile([128, CT, D], F32)

            # ---------------- gates (fp32) ---------------------------------
            with contextlib.ExitStack() as gctx:
                x32_p = gctx.enter_context(tc.tile_pool(name="x32", bufs=4))
                pg = gctx.enter_context(tc.tile_pool(name="pg", bufs=2, space="PSUM"))
                ptrp = gctx.enter_context(tc.tile_pool(name="ptrp", bufs=2, space="PSUM"))
                gT_p = gctx.enter_context(tc.tile_pool(name="gT", bufs=1))
                gT = gT_p.tile([E, TC], F32)

                g_ps = [pg.tile([E, 512], F32, name=f"g_ps{n}") for n in range(NT)]
                for k in range(KD):
                    x32 = x32_p.tile([128, TC], F32, name="x32")
                    nc.sync.dma_start(x32[:], xT_d.ap()[ts(k, 128), :])
                    for n in range(NT):
                        nc.tensor.matmul(
                            g_ps[n][:],
                            lhsT=wg_sb[:, k, :],
                            rhs=x32[:, ts(n, 512)],
                            start=(k == 0),
                            stop=(not has_bg) and k == KD - 1,
                        )
                if has_bg:
                    for n in range(NT):
                        nc.tensor.matmul(
                            g_ps[n][:], lhsT=bg_sb[:1, :], rhs=ones[:1, :],
                            start=False, stop=True,
                        )
                for n in range(NT):
                    nc.vector.tensor_copy(gT[:, ts(n, 512)], g_ps[n][:])
                for c in range(CT):
                    g_tr = ptrp.tile([128, E], F32, name="g_tr")
                    nc.tensor.transpose(g_tr[:], gT[:, ts(c, 128)], ident[:E, :E])
                    nc.vector.tensor_copy(gates[:, c, :], g_tr[:])

            # ---------------- top-2 + softmax ------------------------------
            tk = ctx.enter_context(tc.tile_pool(name="topk", bufs=1))
            m1 = tk.tile([128, CT, 1], F32)
            m2 = tk.tile([128, CT, 1], F32)
            maskA = tk.tile([128, CT, E], F32)
            maskB = tk.tile([128, CT, E], F32)
            g2 = tk.tile([128, CT, E], F32)
            expd = tk.tile([128, CT, 1], F32)
            s1 = tk.tile([128, CT, 1], F32)
            s2 = tk.tile([128, CT, 1], F32)
            selb = tk.tile([128, CT, E], BF16)

            nc.vector.reduce_max(m1[:], gates[:], axis=AX.X)
            nc.vector.tensor_tensor(
                maskA[:], gates[:], m1[:].broadcast_to([128, CT, E]), op=ALU.is_equal
            )
            nc.vector.tensor_scalar(g2[:], maskA[:], NEG_BIG_ := -1.0e30, None, op0=ALU.mult)
            nc.vector.tensor_add(g2[:], g2[:], gates[:])
            nc.vector.reduce_max(m2[:], g2[:], axis=AX.X)
            nc.vector.tensor_tensor(
                maskB[:], g2[:], m2[:].broadcast_to([128, CT, E]), op=ALU.is_equal
            )
            nc.vector.tensor_sub(expd[:], m2[:], m1[:])
            nc.scalar.activation(expd[:], expd[:], AF.Exp)
            nc.vector.tensor_scalar(s1[:], expd[:], 1.0, None, op0=ALU.add)
            nc.vector.reciprocal(s1[:], s1[:])
            nc.vector.tensor_mul(s2[:], expd[:], s1[:])
            nc.vector.tensor_tensor(
                w_sb[:], maskA[:], s1[:].broadcast_to([128, CT, E]), op=ALU.mult
            )
            nc.vector.tensor_tensor(
                g2[:], maskB[:], s2[:].broadcast_to([128, CT, E]), op=ALU.mult
            )
            nc.vector.tensor_add(w_sb[:], w_sb[:], g2[:])
            # sel in g2
            nc.vector.tensor_add(g2[:], maskA[:], maskB[:])
            nc.vector.tensor_copy(selb[:], g2[:])

            # ---------------- prefix positions + pos' ----------------------
            with tc.tile_pool(name="ppre", bufs=4, space="PSUM") as ppre:
                for mc in range(CT):
                    pps = ppre.tile([128, E], F32, name="pps")
                    for kc in range(mc + 1):
                        nc.tensor.matmul(
                            pps[:],
                            lhsT=ones128b[:] if kc < mc else triub[:],
                            rhs=selb[:, kc, :],
                            start=(kc == 0),
                            stop=(kc == mc),
                        )
                    nc.vector.tensor_copy(posp[:, mc, :], pps[:])
            # pos' = pos*sel + sel - 1  (-1 for unselected tokens)
            nc.vector.tensor_mul(posp[:], posp[:], g2[:])
            nc.vector.tensor_add(posp[:], posp[:], g2[:])
            nc.vector.tensor_scalar(posp[:], posp[:], -1.0, None, op0=ALU.add)

            # posT[e, t] via PE transpose (for replicated-pos P^T build)
            with tc.tile_pool(name="ptp", bufs=2, space="PSUM") as ptp:
                for c in range(CT):
                    tpp = ptp.tile([E, 128], F32, name="tpp")
                    nc.tensor.transpose(tpp[:], posp[:, c, :], ident[:])
                    nc.vector.tensor_copy(posT[:, ts(c, 128)], tpp[:])
                for c in range(CT):
                    twp = ptp.tile([E, 128], F32, name="twp", tag="tpp")
                    nc.tensor.transpose(twp[:], w_sb[:, c, :], ident[:])
                    nc.vector.tensor_copy(wT[:, ts(c, 128)], twp[:])

            if has_b2:
                b2r_p = ctx.enter_context(tc.tile_pool(name="b2r", bufs=1))
                b2row_p = ctx.enter_context(tc.tile_pool(name="b2row", bufs=1))
                b2rep = b2r_p.tile([128, E, D], BF16)
                b2row = b2row_p.tile([1, E, D], F32)
                nc.sync.dma_start(b2row[:], b2_d.ap())
                with tc.tile_pool(name="pb2", bufs=2, space="PSUM") as pb2:
                    for e in range(E):
                        for nd in range(ND_):
                            b2ps = pb2.tile([128, 512], F32, name="b2ps")
                            nc.tensor.matmul(
                                b2ps[:], lhsT=ones[:1, :128],
                                rhs=b2row[:, e, ts(nd, 512)],
                                start=True, stop=True,
                            )
                            nc.vector.tensor_copy(b2rep[:, e, ts(nd, 512)], b2ps[:])

            # ---------------- expert loop -----------------------------------
            pe_p = ctx.enter_context(tc.tile_pool(name="pe", bufs=2))      # P_e / P_e^T
            xe_p = ctx.enter_context(tc.tile_pool(name="xe", bufs=2))      # xT_e / y_e
            hT_p = ctx.enter_context(tc.tile_pool(name="hT", bufs=1))
            pr_p = ctx.enter_context(tc.tile_pool(name="posrep", bufs=1))
            w1_p = ctx.enter_context(tc.tile_pool(name="w1", bufs=10))
            w2_p = ctx.enter_context(tc.tile_pool(name="w2", bufs=36))
            pA = ctx.enter_context(tc.tile_pool(name="pA", bufs=3, space="PSUM"))
            pB = ctx.enter_context(tc.tile_pool(name="pB", bufs=4, space="PSUM"))
            pM = ctx.enter_context(tc.tile_pool(name="pM", bufs=1, space="PSUM"))

            for e in range(E):
                # ---- P_e one-hot [t, s] and compact gate weights ----
                Pe = pe_p.tile([128, CT, C], BF16, name="Pe", tag="pe")
                for c in range(CT):
                    nc.vector.tensor_scalar(
                        Pe[:, c, :], iotaC[:], posp[:, c, e : e + 1],
                        None, op0=ALU.is_equal,
                    )
                # ---- gather-as-matmul: xT_e[d, s] = x^T @ P_e ----
                xe = xe_p.tile([128, KD, C], BF16, name="xe", tag="xe")
                for dk in range(KD):
                    for n0, nw in CSL:
                        gps = pA.tile([128, 512], F32, name="gps", tag="pa")
                        for kc in range(CT):
                            nc.tensor.matmul(
                                gps[:, :nw],
                                lhsT=x_sb[:, kc, ts(dk, 128)],
                                rhs=Pe[:, kc, n0 : n0 + nw],
                                start=(kc == 0),
                                stop=(kc == CT - 1),
                            )
                        nc.vector.tensor_copy(xe[:, dk, n0 : n0 + nw], gps[:, :nw])

                # ---- mm1 ----
                hT = hT_p.tile([128, KH, C], BF16, name="hT")
                for mb in range(MB):
                    w1t = []
                    for k in range(KD):
                        t = w1_p.tile([128, 1024], BF16, name="w1t")
                        nc.sync.dma_start(
                            t[:], W1_d.ap()[e, ts(k, 128), ts(mb, 1024)]
                        )
                        w1t.append(t)
                    for mi in range(8):
                        m = mb * 8 + mi
                        for n0, nw in CSL:
                            ps = pA.tile([128, 512], F32, name="ps1", tag="pa")
                            for k in range(KD):
                                nc.tensor.matmul(
                                    ps[:, :nw],
                                    lhsT=w1t[k][:, ts(mi, 128)],
                                    rhs=xe[:, k, n0 : n0 + nw],
                                    start=(k == 0),
                                    stop=(k == KD - 1),
                                )
                            bias = b1_sb[:, e, m : m + 1] if has_b1 else 0.0
                            nc.scalar.activation(
                                hT[:, m, n0 : n0 + nw], ps[:, :nw], AF.Gelu,
                                bias=bias,
                            )

                # ---- mm2 (scaled at eviction) -> y_e bf16 ----
                ye = xe_p.tile([128, MC, D], BF16, name="ye", tag="xe")
                for mc, (s0, sw) in enumerate(MCW):
                    if sw < 128:
                        # pad dead slot rows so the combine matmul sees zeros
                        nc.vector.memset(ye[sw:, mc, :], 0.0)
                for nd in range(ND_):
                    w2t = []
                    for k in range(KH):
                        t = w2_p.tile([128, 512], BF16, name="w2t")
                        nc.sync.dma_start(
                            t[:], W2_d.ap()[e, ts(k, 128), ts(nd, 512)]
                        )
                        w2t.append(t)
                    for mc, (s0, sw) in enumerate(MCW):
                        ps = pB.tile([128, 512], F32, name="ps2", tag="pb")
                        for k in range(KH):
                            nc.tensor.matmul(
                                ps[:sw, :],
                                lhsT=hT[:, k, s0 : s0 + sw],
                                rhs=w2t[k][:],
                                start=(k == 0),
                                stop=(k == KH - 1),
                            )
                        nc.vector.tensor_copy(
                            ye[:sw, mc, ts(nd, 512)], ps[:sw, :]
                        )

                # ---- inverse permutation: out += P_e @ y_e ----
                # P_e^T tiles from replicated pos' (exact integer compare)
                prep = pr_p.tile([128, TC], F32, name="prep")
                wrep = pr_p.tile([128, TC], F32, name="wrep")
                for n in range(NT):
                    pps = pM.tile([128, 512], F32, name="prps", tag="pm")
                    nc.tensor.matmul(
                        pps[:], lhsT=onehot4[:, ts(e, 128)],
                        rhs=posT[:, ts(n, 512)],
                        start=True, stop=True,
                    )
                    nc.vector.tensor_copy(prep[:, ts(n, 512)], pps[:])
                    wps2 = pM.tile([128, 512], F32, name="wps2", tag="pm")
                    nc.tensor.matmul(
                        wps2[:], lhsT=onehot4[:, ts(e, 128)],
                        rhs=wT[:, ts(n, 512)],
                        start=True, stop=True,
                    )
                    nc.vector.tensor_copy(wrep[:, ts(n, 512)], wps2[:])
                PeT = pe_p.tile([128, MC, TC], BF16, name="PeT", tag="pe")
                for mc in range(MC):
                    nc.vector.tensor_scalar(
                        PeT[:, mc, :], prep[:], siota[:, mc : mc + 1],
                        None, op0=ALU.is_equal,
                    )
                    nc.vector.tensor_tensor(
                        PeT[:, mc, :], PeT[:, mc, :], wrep[:], op=ALU.mult
                    )
                for c in range(CT):
                    for nd in range(ND_):
                        ps = pB.tile([128, 512], F32, name="pso", tag="pb")
                        for mc in range(MC):
                            nc.tensor.matmul(
                                ps[:],
                                lhsT=PeT[:, mc, ts(c, 128)],
                                rhs=ye[:, mc, ts(nd, 512)],
                                start=(mc == 0),
                                stop=(mc == MC - 1),
                            )
                        dst = out_sb[:, c, ts(nd, 512)]
                        if e == 0:
                            nc.vector.tensor_copy(dst, ps[:])
                        else:
                            nc.vector.tensor_add(dst, dst, ps[:])
                        if e == E - 1 and not has_b2:
                            nc.sync.dma_start(
                                out_d.ap()[ts(c, 128), ts(nd, 512)], dst
                            )

            # ---- optional b2 term ----
            if has_b2:
                tmp_p = ctx.enter_context(tc.tile_pool(name="tmpb2", bufs=3))
                for c in range(CT):
                    for nd in range(ND_):
                        dst = out_sb[:, c, ts(nd, 512)]
                        for e in range(E):
                            tmp = tmp_p.tile([128, 512], F32, name="b2t")
                            nc.vector.tensor_scalar(
                                tmp[:], b2rep[:, e, ts(nd, 512)],
                                w_sb[:, c, e : e + 1], None, op0=ALU.mult,
                            )
                            nc.vector.tensor_add(dst, dst, tmp[:])

            if has_b2:
                for c in range(CT):
                    nc.sync.dma_start(out_d.ap()[ts(c, 128), :], out_sb[:, c, :])

    nc.compile()
    return nc


def _build_dense_program(has_bg: bool, has_b1: bool, has_b2: bool):
    nc = bacc.Bacc("TRN2", debug=False, num_devices=N_CORES, name="moe_dense")

    xT_d = nc.dram_tensor("xT", [D, TC], F32, kind="ExternalInput")
    Wg_d = nc.dram_tensor("Wg", [D, E], F32, kind="ExternalInput")
    W1_d = nc.dram_tensor("W1", [E, D, H], BF16, kind="ExternalInput")
    W2_d = nc.dram_tensor("W2", [E, H, D], BF16, kind="ExternalInput")
    if has_bg:
        bg_d = nc.dram_tensor("bg", [1, E], F32, kind="ExternalInput")
    if has_b1:
        b1_d = nc.dram_tensor("b1c", [128, E, H // 128], F32, kind="ExternalInput")
    if has_b2:
        b2_d = nc.dram_tensor("b2", [1, E, D], F32, kind="ExternalInput")
    out_d = nc.dram_tensor("out", [TC, D], F32, kind="ExternalOutput")

    KD = D // 128        # 8   k-chunks of D
    KH = H // 128        # 32  k-chunks of H
    NT = TC // 512       # 2   moving-dim slices of tokens
    CT = TC // 128       # 8   token chunks (psum partition tiles)
    ND_ = D // 512       # 2   output D slices
    MB = 4               # mm1 H-chunk blocks (KH/8 per block)

    with tile.TileContext(nc) as tc:
        import contextlib

        with contextlib.ExitStack() as ctx:
            # ---------------- constants / small persistent tiles ------------
            const = ctx.enter_context(tc.tile_pool(name="const", bufs=1))
            ident = const.tile([128, 128], F32)
            make_identity(nc, ident[:])
            ones = const.tile([1, 512], F32)
            nc.vector.memset(ones[:], 1.0)
            wg_sb = const.tile([128, KD, E], F32)
            nc.sync.dma_start(
                wg_sb[:], Wg_d.ap().rearrange("(k p) e -> p k e", p=128)
            )
            if has_bg:
                bg_sb = const.tile([1, E], F32)
                nc.sync.dma_start(bg_sb[:], bg_d.ap())
            if has_b1:
                b1_sb = const.tile([128, E, KH], F32)
                nc.sync.dma_start(b1_sb[:], b1_d.ap())

            small = ctx.enter_context(tc.tile_pool(name="small", bufs=1))
            gates = small.tile([128, CT, E], F32)
            w_sb = small.tile([128, CT, E], F32)

            # persistent big tensors
            xbf_p = ctx.enter_context(tc.tile_pool(name="xbf", bufs=1))
            xbf = xbf_p.tile([128, KD, TC], BF16)
            out_p = ctx.enter_context(tc.tile_pool(name="outsb", bufs=1))
            out_sb = out_p.tile([128, CT, D], F32)
            hT_p = ctx.enter_context(tc.tile_pool(name="hT", bufs=1))

            # ---------------- gates (fp32) + x cast --------------------------
            with contextlib.ExitStack() as gctx:
                x32_p = gctx.enter_context(tc.tile_pool(name="x32", bufs=3))
                pg = gctx.enter_context(
                    tc.tile_pool(name="pg", bufs=2, space="PSUM")
                )
                ptr = gctx.enter_context(
                    tc.tile_pool(name="ptr", bufs=2, space="PSUM")
                )
                gT_p = gctx.enter_context(tc.tile_pool(name="gT", bufs=1))
                gT = gT_p.tile([E, TC], F32)

                g_ps = [pg.tile([E, 512], F32, name=f"g_ps{n}") for n in range(NT)]
                for k in range(KD):
                    x32 = x32_p.tile([128, TC], F32, name="x32")
                    nc.sync.dma_start(x32[:], xT_d.ap()[ts(k, 128), :])
                    nc.vector.tensor_copy(xbf[:, k, :], x32[:])
                    for n in range(NT):
                        nc.tensor.matmul(
                            g_ps[n][:],
                            lhsT=wg_sb[:, k, :],
                            rhs=x32[:, ts(n, 512)],
                            start=(k == 0),
                            stop=(not has_bg) and k == KD - 1,
                        )
                if has_bg:
                    for n in range(NT):
                        nc.tensor.matmul(
                            g_ps[n][:],
                            lhsT=bg_sb[:1, :],
                            rhs=ones[:1, :],
                            start=False,
                            stop=True,
                        )
                for n in range(NT):
                    nc.vector.tensor_copy(gT[:, ts(n, 512)], g_ps[n][:])

                # transpose [E, T] -> [T-chunk, E] tiles
                for c in range(CT):
                    g_tr = ptr.tile([128, E], F32, name="g_tr")
                    nc.tensor.transpose(
                        g_tr[:], gT[:, ts(c, 128)], ident[:E, :E]
                    )
                    nc.vector.tensor_copy(gates[:, c, :], g_tr[:])

            # ---------------- top-2 + softmax -> w ---------------------------
            tk = ctx.enter_context(tc.tile_pool(name="topk", bufs=1))
            m1 = tk.tile([128, CT, 1], F32)
            m2 = tk.tile([128, CT, 1], F32)
            mask1 = tk.tile([128, CT, E], F32)
            mask2 = tk.tile([128, CT, E], F32)
            g2 = tk.tile([128, CT, E], F32)
            expd = tk.tile([128, CT, 1], F32)
            s1 = tk.tile([128, CT, 1], F32)
            s2 = tk.tile([128, CT, 1], F32)

            nc.vector.reduce_max(m1[:], gates[:], axis=AX.X)
            nc.vector.tensor_tensor(
                mask1[:], gates[:], m1[:].broadcast_to([128, CT, E]), op=ALU.is_equal
            )
            nc.vector.tensor_scalar(g2[:], mask1[:], NEG_BIG, None, op0=ALU.mult)
            nc.vector.tensor_add(g2[:], g2[:], gates[:])
            nc.vector.reduce_max(m2[:], g2[:], axis=AX.X)
            nc.vector.tensor_tensor(
                mask2[:], g2[:], m2[:].broadcast_to([128, CT, E]), op=ALU.is_equal
            )
            nc.vector.tensor_sub(expd[:], m2[:], m1[:])
            nc.scalar.activation(expd[:], expd[:], AF.Exp)
            nc.vector.tensor_scalar(s1[:], expd[:], 1.0, None, op0=ALU.add)
            nc.vector.reciprocal(s1[:], s1[:])
            nc.vector.tensor_mul(s2[:], expd[:], s1[:])
            nc.vector.tensor_tensor(
                mask1[:], mask1[:], s1[:].broadcast_to([128, CT, E]), op=ALU.mult
            )
            nc.vector.tensor_tensor(
                mask2[:], mask2[:], s2[:].broadcast_to([128, CT, E]), op=ALU.mult
            )
            nc.vector.tensor_add(w_sb[:], mask1[:], mask2[:])

            # optional b2 row replicated across partitions (via ones outer-prod)
            if has_b2:
                b2r_p = ctx.enter_context(tc.tile_pool(name="b2r", bufs=1))
                b2row_p = ctx.enter_context(tc.tile_pool(name="b2row", bufs=1))
                b2rep = b2r_p.tile([128, E, D], BF16)
                b2row = b2row_p.tile([1, E, D], F32)
                nc.sync.dma_start(b2row[:], b2_d.ap())
                with tc.tile_pool(name="pb2", bufs=2, space="PSUM") as pb2:
                    for e in range(E):
                        for nd in range(ND_):
                            b2ps = pb2.tile([128, 512], F32, name="b2ps")
                            nc.tensor.matmul(
                                b2ps[:],
                                lhsT=ones[:1, :128],
                                rhs=b2row[:, e, ts(nd, 512)],
                                start=True,
                                stop=True,
                            )
                            nc.vector.tensor_copy(b2rep[:, e, ts(nd, 512)], b2ps[:])

            # ---------------- expert loop ------------------------------------
            w1_p = ctx.enter_context(tc.tile_pool(name="w1", bufs=10))
            w2_p = ctx.enter_context(tc.tile_pool(name="w2", bufs=34))
            pmm1 = ctx.enter_context(tc.tile_pool(name="pmm1", bufs=4, space="PSUM"))
            pmm2 = ctx.enter_context(tc.tile_pool(name="pmm2", bufs=3, space="PSUM"))
            tmp_p = ctx.enter_context(tc.tile_pool(name="tmp", bufs=3))

            for e in range(E):
                hT = hT_p.tile([128, KH, TC], BF16, name="hT")

                # ---- mm1: hT = gelu(W1[e].T @ xT + b1[e]) ----
                for mb in range(MB):
                    w1t = []
                    for k in range(KD):
                        t = w1_p.tile([128, 1024], BF16, name="w1t")
                        nc.sync.dma_start(
                            t[:], W1_d.ap()[e, ts(k, 128), ts(mb, 1024)]
                        )
                        w1t.append(t)
                    for mi in range(8):
                        m = mb * 8 + mi
                        for n in range(NT):
                            ps = pmm1.tile([128, 512], F32, name="ps1")
                            for k in range(KD):
                                nc.tensor.matmul(
                                    ps[:],
                                    lhsT=w1t[k][:, ts(mi, 128)],
                                    rhs=xbf[:, k, ts(n, 512)],
                                    start=(k == 0),
                                    stop=(k == KD - 1),
                                )
                            bias = b1_sb[:, e, m : m + 1] if has_b1 else 0.0
                            nc.scalar.activation(
                                hT[:, m, ts(n, 512)], ps[:], AF.Gelu, bias=bias
                            )

                # ---- mm2: out += w[:, e] * (hT.T @ W2[e]) ----
                for nd in range(ND_):
                    w2t = []
                    for k in range(KH):
                        t = w2_p.tile([128, 512], BF16, name="w2t")
                        nc.sync.dma_start(
                            t[:], W2_d.ap()[e, ts(k, 128), ts(nd, 512)]
                        )
                        w2t.append(t)
                    for c in range(CT):
                        ps = pmm2.tile([128, 512], F32, name="ps2")
                        for k in range(KH):
                            nc.tensor.matmul(
                                ps[:],
                                lhsT=hT[:, k, ts(c, 128)],
                                rhs=w2t[k][:],
                                start=(k == 0),
                                stop=(k == KH - 1),
                            )
                        wv = w_sb[:, c, e : e + 1]
                        dst = out_sb[:, c, ts(nd, 512)]
                        if e == 0:
                            nc.vector.tensor_scalar(dst, ps[:], wv, None, op0=ALU.mult)
                        else:
                            tmp = tmp_p.tile([128, 512], F32, name="acc_tmp")
                            nc.vector.tensor_scalar(
                                tmp[:], ps[:], wv, None, op0=ALU.mult
                            )
                            nc.vector.tensor_add(dst, dst, tmp[:])

            # ---- optional b2 term: out += w @ b2 ----
            if has_b2:
                for c in range(CT):
                    for nd in range(ND_):
                        dst = out_sb[:, c, ts(nd, 512)]
                        for e in range(E):
                            tmp = tmp_p.tile([128, 512], F32, name="acc_tmp")
                            nc.vector.tensor_scalar(
                                tmp[:],
                                b2rep[:, e, ts(nd, 512)],
                                w_sb[:, c, e : e + 1],
                                None,
                                op0=ALU.mult,
                            )
                            nc.vector.tensor_add(dst, dst, tmp[:])

            # ---- store ----
            for c in range(CT):
                nc.sync.dma_start(out_d.ap()[ts(c, 128), :], out_sb[:, c, :])

    nc.compile()
    return nc


def _capacity_ok(xf, Wg, bg):
    """Host-side check only: if any (core, expert) routed count could exceed
    the compiled capacity C, fall back to the dense program (all math still
    happens on device either way)."""
    g = xf @ Wg + bg
    top2 = np.argpartition(-g, 2, axis=1)[:, :2]
    for c in range(N_CORES):
        t = top2[c * TC : (c + 1) * TC]
        for e in range(E):
            if (t == e).sum() > C - 8:
                return False
    return True


def kernel(x, Wg, bg, W1, b1, W2, b2):
    global LAST_EXEC_NS
    import os

    x = np.asarray(x, dtype=np.float32)
    Wg = np.asarray(Wg, dtype=np.float32)
    bg = np.asarray(bg, dtype=np.float32)
    W1 = np.asarray(W1, dtype=np.float32)
    b1 = np.asarray(b1, dtype=np.float32)
    W2 = np.asarray(W2, dtype=np.float32)
    b2 = np.asarray(b2, dtype=np.float32)

    B, S, _ = x.shape
    xf = x.reshape(-1, D)
    assert xf.shape[0] == N_CORES * TC

    has_bg = bool(np.any(bg))
    has_b1 = bool(np.any(b1))
    has_b2 = bool(np.any(b2))
    routed = _capacity_ok(xf, Wg, bg)
    key = (routed, has_bg, has_b1, has_b2)
    if key not in _PROGRAMS:
        build = _build_routed_program if routed else _build_dense_program
        _PROGRAMS[key] = build(has_bg, has_b1, has_b2)
    nc = _PROGRAMS[key]

    W1bf = W1.astype(ml_dtypes.bfloat16)
    W2bf = W2.astype(ml_dtypes.bfloat16)

    in_maps = []
    for c in range(N_CORES):
        shard = xf[c * TC : (c + 1) * TC]
        m = {
            "xT": np.ascontiguousarray(shard.T),
            "Wg": Wg,
            "W1": W1bf,
            "W2": W2bf,
        }
        if routed:
            m["xrows"] = shard.astype(ml_dtypes.bfloat16)
        if has_bg:
            m["bg"] = bg.reshape(1, E)
        if has_b1:
            m["b1c"] = np.ascontiguousarray(
                b1.reshape(E, H // 128, 128).transpose(2, 0, 1)
            )
        if has_b2:
            m["b2"] = b2.reshape(1, E, D)
        in_maps.append(m)

    trace = bool(os.environ.get("KERNEL_TRACE"))
    res = run_bass_kernel_spmd(
        nc, in_maps, core_ids=list(range(N_CORES)), trace=trace
    )
    LAST_EXEC_NS = res.exec_time_ns

    out = np.concatenate([res.results[c]["out"] for c in range(N_CORES)], axis=0)
    return out.reshape(B, S, D)


# revision 23
# speedup vs baseline: 1.0102x; 1.0078x over previous
"""Routed (top-2) MoE via permutation MATMULS — no indirect DMA.

Indirect DMA on trn2 costs ~12us per 128 scattered rows (descriptor-bound,
single SWDGE queue) and same-tensor scatters serialize, so gather/scatter
routing drowns in DMA time.  Instead the token->slot permutation is applied
on the TensorEngine:

  pos[t,e]  exclusive prefix count of selected tokens (triangular matmul)
  pos' = sel ? pos : -1
  P_e[t,s]  = (pos'[t,e] == s)        one-hot [T, C] built by DVE is_equal
  x_e^T     = x^T @ P_e               gather as matmul (exact 0/1 weights)
  w_c       = P_e^T @ w[:,e]          compact per-slot gate weights
  h_e^T     = gelu(W1[e]^T @ x_e^T)   routed mm1 (bf16, C=640 slots)
  y_e       = w_c * (h_e^T)^T @ W2[e] routed mm2, scaled at PSUM eviction
  out      += P_e @ y_e               inverse permutation as matmul, P_e^T
                                      tiles from a replicated-pos is_equal
"""

import sys

if "/opt/trn_rl_repo" not in sys.path:
    sys.path.insert(0, "/opt/trn_rl_repo")

import contextlib

import numpy as np
import ml_dtypes

import concourse.bacc as bacc
import concourse.bass as bass
import concourse.mybir as mybir
import concourse.tile as tile
from concourse.bass import ts
from concourse.bass_utils import run_bass_kernel_spmd
from concourse.masks import make_identity

AF = mybir.ActivationFunctionType
ALU = mybir.AluOpType
AX = mybir.AxisListType
F32 = mybir.dt.float32
BF16 = mybir.dt.bfloat16
I32 = mybir.dt.int32

N_CORES = 8
D = 1024
H = 4096
E = 4
TC = 1024
C = 576                      # per-expert slot capacity (max observed 553)

KD = D // 128                # 8
KH = H // 128                # 32
NT = TC // 512               # 2
CT = TC // 128               # 8
ND_ = D // 512               # 2
MB = 4
MC = (C + 127) // 128        # 5 slot chunks (last one 64 wide)
MCW = [(mc * 128, min(128, C - mc * 128)) for mc in range(MC)]
CSL = [(0, 288), (288, 288)] # even moving-dim slices of C (less MM overhead)

NEG_BIG = -1.0e30
_PROGRAMS = {}
LAST_EXEC_NS = None


def _build_routed_program(has_bg: bool, has_b1: bool, has_b2: bool):
    nc = bacc.Bacc("TRN2", debug=False, num_devices=N_CORES, name="moe_perm")

    xT_d = nc.dram_tensor("xT", [D, TC], F32, kind="ExternalInput")
    xr_d = nc.dram_tensor("xrows", [TC, D], BF16, kind="ExternalInput")
    Wg_d = nc.dram_tensor("Wg", [D, E], F32, kind="ExternalInput")
    W1_d = nc.dram_tensor("W1", [E, D, H], BF16, kind="ExternalInput")
    W2_d = nc.dram_tensor("W2", [E, H, D], BF16, kind="ExternalInput")
    if has_bg:
        bg_d = nc.dram_tensor("bg", [1, E], F32, kind="ExternalInput")
    if has_b1:
        b1_d = nc.dram_tensor("b1c", [128, E, KH], F32, kind="ExternalInput")
    if has_b2:
        b2_d = nc.dram_tensor("b2", [1, E, D], F32, kind="ExternalInput")
    out_d = nc.dram_tensor("out", [TC, D], F32, kind="ExternalOutput")

    with tile.TileContext(nc) as tc:
        with contextlib.ExitStack() as ctx:
            # ---------------- constants ------------------------------------
            const = ctx.enter_context(tc.tile_pool(name="const", bufs=1))
            ident = const.tile([128, 128], F32)
            make_identity(nc, ident[:])
            ones = const.tile([1, 512], F32)
            nc.vector.memset(ones[:], 1.0)
            ones128b = const.tile([128, 128], BF16)
            nc.vector.memset(ones128b[:], 1.0)
            # strict upper triangular (i < j) for exclusive prefix counts
            it_row = const.tile([128, 1], I32)
            nc.gpsimd.iota(it_row[:], pattern=[[1, 1]], base=0, channel_multiplier=1)
            it_col = const.tile([128, 128], I32)
            nc.gpsimd.iota(it_col[:], pattern=[[1, 128]], base=0, channel_multiplier=0)
            triub = const.tile([128, 128], BF16)
            trif = const.tile([128, 128], F32)
            nc.vector.tensor_tensor(
                trif[:], it_row[:].broadcast_to([128, 128]), it_col[:], op=ALU.is_lt
            )
            nc.vector.tensor_copy(triub[:], trif[:])
            # slot-index rows / per-partition slot ids for one-hot builds
            iotaC_i = const.tile([128, C], I32)
            nc.gpsimd.iota(iotaC_i[:], pattern=[[1, C]], base=0, channel_multiplier=0)
            iotaC = const.tile([128, C], F32)
            nc.vector.tensor_copy(iotaC[:], iotaC_i[:])
            siota = const.tile([128, MC], F32)
            rowf = const.tile([128, 1], F32)
            nc.vector.tensor_copy(rowf[:], it_row[:])
            for mc in range(MC):
                nc.vector.tensor_scalar(
                    siota[:, mc : mc + 1], rowf[:], float(mc * 128), None, op0=ALU.add
                )
            # onehot4[:, e*128:(e+1)*128] has row e all-ones (K=4 selector for
            # replicating posT row e across 128 partitions via matmul)
            onehot4 = const.tile([4, E * 128], F32)
            for e in range(E):
                nc.vector.tensor_scalar(
                    onehot4[:, ts(e, 128)], rowf[:4, :].broadcast_to([4, 128]),
                    float(e), None, op0=ALU.is_equal,
                )
            wg_sb = const.tile([128, KD, E], F32)
            nc.sync.dma_start(
                wg_sb[:], Wg_d.ap().rearrange("(k p) e -> p k e", p=128)
            )
            if has_bg:
                bg_sb = const.tile([1, E], F32)
                nc.sync.dma_start(bg_sb[:], bg_d.ap())
            if has_b1:
                b1_sb = const.tile([128, E, KH], F32)
                nc.sync.dma_start(b1_sb[:], b1_d.ap())

            small = ctx.enter_context(tc.tile_pool(name="small", bufs=1))
            gates = small.tile([128, CT, E], F32)
            w_sb = small.tile([128, CT, E], F32)
            posp = small.tile([128, CT, E], F32)    # pos' = sel ? pos : -1
            posT = small.tile([E, TC], F32)
            wT = small.tile([E, TC], F32)

            # x rows (bf16) resident: lhsT tiles for the gather matmul
            xs_p = ctx.enter_context(tc.tile_pool(name="xsb", bufs=1))
            x_sb = xs_p.tile([128, CT, D], BF16)
            nc.sync.dma_start(
                x_sb[:], xr_d.ap().rearrange("(c p) d -> p c d", p=128)
            )
            out_p = ctx.enter_context(tc.tile_pool(name="outsb", bufs=1))
            out_sb = out_p.tile([128, CT, D], F32)

            # ---------------- gates (fp32) ---------------------------------
            with contextlib.ExitStack() as gctx:
                x32_p = gctx.enter_context(tc.tile_pool(name="x32", bufs=3))
                pg = gctx.enter_context(tc.tile_pool(name="pg", bufs=2, space="PSUM"))
                ptrp = gctx.enter_context(tc.tile_pool(name="ptrp", bufs=2, space="PSUM"))
                gT_p = gctx.enter_context(tc.tile_pool(name="gT", bufs=1))
                gT = gT_p.tile([E, TC], F32)

                g_ps = [pg.tile([E, 512], F32, name=f"g_ps{n}") for n in range(NT)]
                for k in range(KD):
                    x32 = x32_p.tile([128, TC], F32, name="x32")
                    nc.sync.dma_start(x32[:], xT_d.ap()[ts(k, 128), :])
                    for n in range(NT):
                        nc.tensor.matmul(
                            g_ps[n][:],
                            lhsT=wg_sb[:, k, :],
                            rhs=x32[:, ts(n, 512)],
                            start=(k == 0),
                            stop=(not has_bg) and k == KD - 1,
                        )
                if has_bg:
                    for n in range(NT):
                        nc.tensor.matmul(
                            g_ps[n][:], lhsT=bg_sb[:1, :], rhs=ones[:1, :],
                            start=False, stop=True,
                        )
                for n in range(NT):
                    nc.vector.tensor_copy(gT[:, ts(n, 512)], g_ps[n][:])
                for c in range(CT):
                    g_tr = ptrp.tile([128, E], F32, name="g_tr")
                    nc.tensor.transpose(g_tr[:], gT[:, ts(c, 128)], ident[:E, :E])
                    nc.vector.tensor_copy(gates[:, c, :], g_tr[:])

            # ---------------- top-2 + softmax ------------------------------
            tk = ctx.enter_context(tc.tile_pool(name="topk", bufs=1))
            m1 = tk.tile([128, CT, 1], F32)
            m2 = tk.tile([128, CT, 1], F32)
            maskA = tk.tile([128, CT, E], F32)
            maskB = tk.tile([128, CT, E], F32)
            g2 = tk.tile([128, CT, E], F32)
            expd = tk.tile([128, CT, 1], F32)
            s1 = tk.tile([128, CT, 1], F32)
            s2 = tk.tile([128, CT, 1], F32)
            selb = tk.tile([128, CT, E], BF16)

            nc.vector.reduce_max(m1[:], gates[:], axis=AX.X)
            nc.vector.tensor_tensor(
                maskA[:], gates[:], m1[:].broadcast_to([128, CT, E]), op=ALU.is_equal
            )
            nc.vector.tensor_scalar(g2[:], maskA[:], NEG_BIG_ := -1.0e30, None, op0=ALU.mult)
            nc.vector.tensor_add(g2[:], g2[:], gates[:])
            nc.vector.reduce_max(m2[:], g2[:], axis=AX.X)
            nc.vector.tensor_tensor(
                maskB[:], g2[:], m2[:].broadcast_to([128, CT, E]), op=ALU.is_equal
            )
            nc.vector.tensor_sub(expd[:], m2[:], m1[:])
            nc.scalar.activation(expd[:], expd[:], AF.Exp)
            nc.vector.tensor_scalar(s1[:], expd[:], 1.0, None, op0=ALU.add)
            nc.vector.reciprocal(s1[:], s1[:])
            nc.vector.tensor_mul(s2[:], expd[:], s1[:])
            nc.vector.tensor_tensor(
                w_sb[:], maskA[:], s1[:].broadcast_to([128, CT, E]), op=ALU.mult
            )
            nc.vector.tensor_tensor(
                g2[:], maskB[:], s2[:].broadcast_to([128, CT, E]), op=ALU.mult
            )
            nc.vector.tensor_add(w_sb[:], w_sb[:], g2[:])
            # sel in g2
            nc.vector.tensor_add(g2[:], maskA[:], maskB[:])
            nc.vector.tensor_copy(selb[:], g2[:])

            # ---------------- prefix positions + pos' ----------------------
            with tc.tile_pool(name="ppre", bufs=4, space="PSUM") as ppre:
                for mc in range(CT):
                    pps = ppre.tile([128, E], F32, name="pps")
                    for kc in range(mc + 1):
                        nc.tensor.matmul(
                            pps[:],
                            lhsT=ones128b[:] if kc < mc else triub[:],
                            rhs=selb[:, kc, :],
                            start=(kc == 0),
                            stop=(kc == mc),
                        )
                    nc.vector.tensor_copy(posp[:, mc, :], pps[:])
            # pos' = pos*sel + sel - 1  (-1 for unselected tokens)
            nc.vector.tensor_mul(posp[:], posp[:], g2[:])
            nc.vector.tensor_add(posp[:], posp[:], g2[:])
            nc.vector.tensor_scalar(posp[:], posp[:], -1.0, None, op0=ALU.add)

            # posT[e, t] via PE transpose (for replicated-pos P^T build)
            with tc.tile_pool(name="ptp", bufs=2, space="PSUM") as ptp:
                for c in range(CT):
                    tpp = ptp.tile([E, 128], F32, name="tpp")
                    nc.tensor.transpose(tpp[:], posp[:, c, :], ident[:])
                    nc.vector.tensor_copy(posT[:, ts(c, 128)], tpp[:])
                for c in range(CT):
                    twp = ptp.tile([E, 128], F32, name="twp", tag="tpp")
                    nc.tensor.transpose(twp[:], w_sb[:, c, :], ident[:])
                    nc.vector.tensor_copy(wT[:, ts(c, 128)], twp[:])

            if has_b2:
                b2r_p = ctx.enter_context(tc.tile_pool(name="b2r", bufs=1))
                b2row_p = ctx.enter_context(tc.tile_pool(name="b2row", bufs=1))
                b2rep = b2r_p.tile([128, E, D], BF16)
                b2row = b2row_p.tile([1, E, D], F32)
                nc.sync.dma_start(b2row[:], b2_d.ap())
                with tc.tile_pool(name="pb2", bufs=2, space="PSUM") as pb2:
                    for e in range(E):
                        for nd in range(ND_):
                            b2ps = pb2.tile([128, 512], F32, name="b2ps")
                            nc.tensor.matmul(
                                b2ps[:], lhsT=ones[:1, :128],
                                rhs=b2row[:, e, ts(nd, 512)],
                                start=True, stop=True,
                            )
                            nc.vector.tensor_copy(b2rep[:, e, ts(nd, 512)], b2ps[:])

            # ---------------- expert loop -----------------------------------
            pe_p = ctx.enter_context(tc.tile_pool(name="pe", bufs=2))      # P_e / P_e^T
            xe_p = ctx.enter_context(tc.tile_pool(name="xe", bufs=2))      # xT_e / y_e
            hT_p = ctx.enter_context(tc.tile_pool(name="hT", bufs=1))
            pr_p = ctx.enter_context(tc.tile_pool(name="posrep", bufs=1))
            w1_p = ctx.enter_context(tc.tile_pool(name="w1", bufs=10))
            w2_p = ctx.enter_context(tc.tile_pool(name="w2", bufs=36))
            pA = ctx.enter_context(tc.tile_pool(name="pA", bufs=3, space="PSUM"))
            pB = ctx.enter_context(tc.tile_pool(name="pB", bufs=3, space="PSUM"))
            pM = ctx.enter_context(tc.tile_pool(name="pM", bufs=2, space="PSUM"))

            for e in range(E):
                # ---- P_e one-hot [t, s] and compact gate weights ----
                Pe = pe_p.tile([128, CT, C], BF16, name="Pe", tag="pe")
                for c in range(CT):
                    nc.vector.tensor_scalar(
                        Pe[:, c, :], iotaC[:], posp[:, c, e : e + 1],
                        None, op0=ALU.is_equal,
                    )
                # ---- gather-as-matmul: xT_e[d, s] = x^T @ P_e ----
                xe = xe_p.tile([128, KD, C], BF16, name="xe", tag="xe")
                for dk in range(KD):
                    for n0, nw in CSL:
                        gps = pA.tile([128, 512], F32, name="gps", tag="pa")
                        for kc in range(CT):
                            nc.tensor.matmul(
                                gps[:, :nw],
                                lhsT=x_sb[:, kc, ts(dk, 128)],
                                rhs=Pe[:, kc, n0 : n0 + nw],
                                start=(kc == 0),
                                stop=(kc == CT - 1),
                            )
                        nc.vector.tensor_copy(xe[:, dk, n0 : n0 + nw], gps[:, :nw])

                # ---- mm1 ----
                hT = hT_p.tile([128, KH, C], BF16, name="hT")
                for mb in range(MB):
                    w1t = []
                    for k in range(KD):
                        t = w1_p.tile([128, 1024], BF16, name="w1t")
                        nc.sync.dma_start(
                            t[:], W1_d.ap()[e, ts(k, 128), ts(mb, 1024)]
                        )
                        w1t.append(t)
                    for mi in range(8):
                        m = mb * 8 + mi
                        for n0, nw in CSL:
                            ps = pA.tile([128, 512], F32, name="ps1", tag="pa")
                            for k in range(KD):
                                nc.tensor.matmul(
                                    ps[:, :nw],
                                    lhsT=w1t[k][:, ts(mi, 128)],
                                    rhs=xe[:, k, n0 : n0 + nw],
                                    start=(k == 0),
                                    stop=(k == KD - 1),
                                )
                            bias = b1_sb[:, e, m : m + 1] if has_b1 else 0.0
                            nc.scalar.activation(
                                hT[:, m, n0 : n0 + nw], ps[:, :nw], AF.Gelu,
                                bias=bias,
                            )

                # ---- mm2 (scaled at eviction) -> y_e bf16 ----
                ye = xe_p.tile([128, MC, D], BF16, name="ye", tag="xe")
                for mc, (s0, sw) in enumerate(MCW):
                    if sw < 128:
                        # pad dead slot rows so the combine matmul sees zeros
                        nc.vector.memset(ye[sw:, mc, :], 0.0)
                for nd in range(ND_):
                    w2t = []
                    for k in range(KH):
                        t = w2_p.tile([128, 512], BF16, name="w2t")
                        nc.sync.dma_start(
                            t[:], W2_d.ap()[e, ts(k, 128), ts(nd, 512)]
                        )
                        w2t.append(t)
                    for mc, (s0, sw) in enumerate(MCW):
                        ps = pB.tile([128, 512], F32, name="ps2", tag="pb")
                        for k in range(KH):
                            nc.tensor.matmul(
                                ps[:sw, :],
                                lhsT=hT[:, k, s0 : s0 + sw],
                                rhs=w2t[k][:],
                                start=(k == 0),
                                stop=(k == KH - 1),
                            )
                        nc.vector.tensor_copy(
                            ye[:sw, mc, ts(nd, 512)], ps[:sw, :]
                        )

                # ---- inverse permutation: out += P_e @ y_e ----
                # P_e^T tiles from replicated pos' (exact integer compare)
                prep = pr_p.tile([128, TC], F32, name="prep")
                wrep = pr_p.tile([128, TC], F32, name="wrep")
                for n in range(NT):
                    pps = pM.tile([128, 512], F32, name="prps", tag="pm")
                    nc.tensor.matmul(
                        pps[:], lhsT=onehot4[:, ts(e, 128)],
                        rhs=posT[:, ts(n, 512)],
                        start=True, stop=True,
                    )
                    nc.vector.tensor_copy(prep[:, ts(n, 512)], pps[:])
                    wps2 = pM.tile([128, 512], F32, name="wps2", tag="pm")
                    nc.tensor.matmul(
                        wps2[:], lhsT=onehot4[:, ts(e, 128)],
                        rhs=wT[:, ts(n, 512)],
                        start=True, stop=True,
                    )
                    nc.vector.tensor_copy(wrep[:, ts(n, 512)], wps2[:])
                PeT = pe_p.tile([128, MC, TC], BF16, name="PeT", tag="pe")
                for mc in range(MC):
                    nc.vector.tensor_scalar(
                        PeT[:, mc, :], prep[:], siota[:, mc : mc + 1],
                        None, op0=ALU.is_equal,
                    )
                    nc.vector.tensor_tensor(
                        PeT[:, mc, :], PeT[:, mc, :], wrep[:], op=ALU.mult
                    )
                for c in range(CT):
                    for nd in range(ND_):
                        ps = pB.tile([128, 512], F32, name="pso", tag="pb")
                        for mc in range(MC):
                            nc.tensor.matmul(
                                ps[:],
                                lhsT=PeT[:, mc, ts(c, 128)],
                                rhs=ye[:, mc, ts(nd, 512)],
                                start=(mc == 0),
                                stop=(mc == MC - 1),
                            )
                        dst = out_sb[:, c, ts(nd, 512)]
                        if e == 0:
                            nc.vector.tensor_copy(dst, ps[:])
                        else:
                            nc.vector.tensor_add(dst, dst, ps[:])
                        if e == E - 1 and not has_b2:
                            nc.sync.dma_start(
                                out_d.ap()[ts(c, 128), ts(nd, 512)], dst
                            )

            # ---- optional b2 term ----
            if has_b2:
                tmp_p = ctx.enter_context(tc.tile_pool(name="tmpb2", bufs=3))
                for c in range(CT):
                    for nd in range(ND_):
                        dst = out_sb[:, c, ts(nd, 512)]
                        for e in range(E):
                            tmp = tmp_p.tile([128, 512], F32, name="b2t")
                            nc.vector.tensor_scalar(
                                tmp[:], b2rep[:, e, ts(nd, 512)],
                                w_sb[:, c, e : e + 1], None, op0=ALU.mult,
                            )
                            nc.vector.tensor_add(dst, dst, tmp[:])

            if has_b2:
                for c in range(CT):
                    nc.sync.dma_start(out_d.ap()[ts(c, 128), :], out_sb[:, c, :])

    nc.compile()
    return nc


def _build_dense_program(has_bg: bool, has_b1: bool, has_b2: bool):
    nc = bacc.Bacc("TRN2", debug=False, num_devices=N_CORES, name="moe_dense")

    xT_d = nc.dram_tensor("xT", [D, TC], F32, kind="ExternalInput")
    Wg_d = nc.dram_tensor("Wg", [D, E], F32, kind="ExternalInput")
    W1_d = nc.dram_tensor("W1", [E, D, H], BF16, kind="ExternalInput")
    W2_d = nc.dram_tensor("W2", [E, H, D], BF16, kind="ExternalInput")
    if has_bg:
        bg_d = nc.dram_tensor("bg", [1, E], F32, kind="ExternalInput")
    if has_b1:
        b1_d = nc.dram_tensor("b1c", [128, E, H // 128], F32, kind="ExternalInput")
    if has_b2:
        b2_d = nc.dram_tensor("b2", [1, E, D], F32, kind="ExternalInput")
    out_d = nc.dram_tensor("out", [TC, D], F32, kind="ExternalOutput")

    KD = D // 128        # 8   k-chunks of D
    KH = H // 128        # 32  k-chunks of H
    NT = TC // 512       # 2   moving-dim slices of tokens
    CT = TC // 128       # 8   token chunks (psum partition tiles)
    ND_ = D // 512       # 2   output D slices
    MB = 4               # mm1 H-chunk blocks (KH/8 per block)

    with tile.TileContext(nc) as tc:
        import contextlib

        with contextlib.ExitStack() as ctx:
            # ---------------- constants / small persistent tiles ------------
            const = ctx.enter_context(tc.tile_pool(name="const", bufs=1))
            ident = const.tile([128, 128], F32)
            make_identity(nc, ident[:])
            ones = const.tile([1, 512], F32)
            nc.vector.memset(ones[:], 1.0)
            wg_sb = const.tile([128, KD, E], F32)
            nc.sync.dma_start(
                wg_sb[:], Wg_d.ap().rearrange("(k p) e -> p k e", p=128)
            )
            if has_bg:
                bg_sb = const.tile([1, E], F32)
                nc.sync.dma_start(bg_sb[:], bg_d.ap())
            if has_b1:
                b1_sb = const.tile([128, E, KH], F32)
                nc.sync.dma_start(b1_sb[:], b1_d.ap())

            small = ctx.enter_context(tc.tile_pool(name="small", bufs=1))
            gates = small.tile([128, CT, E], F32)
            w_sb = small.tile([128, CT, E], F32)

            # persistent big tensors
            xbf_p = ctx.enter_context(tc.tile_pool(name="xbf", bufs=1))
            xbf = xbf_p.tile([128, KD, TC], BF16)
            out_p = ctx.enter_context(tc.tile_pool(name="outsb", bufs=1))
            out_sb = out_p.tile([128, CT, D], F32)
            hT_p = ctx.enter_context(tc.tile_pool(name="hT", bufs=1))

            # ---------------- gates (fp32) + x cast --------------------------
            with contextlib.ExitStack() as gctx:
                x32_p = gctx.enter_context(tc.tile_pool(name="x32", bufs=3))
                pg = gctx.enter_context(
                    tc.tile_pool(name="pg", bufs=2, space="PSUM")
                )
                ptr = gctx.enter_context(
                    tc.tile_pool(name="ptr", bufs=2, space="PSUM")
                )
                gT_p = gctx.enter_context(tc.tile_pool(name="gT", bufs=1))
                gT = gT_p.tile([E, TC], F32)

                g_ps = [pg.tile([E, 512], F32, name=f"g_ps{n}") for n in range(NT)]
                for k in range(KD):
                    x32 = x32_p.tile([128, TC], F32, name="x32")
                    nc.sync.dma_start(x32[:], xT_d.ap()[ts(k, 128), :])
                    nc.vector.tensor_copy(xbf[:, k, :], x32[:])
                    for n in range(NT):
                        nc.tensor.matmul(
                            g_ps[n][:],
                            lhsT=wg_sb[:, k, :],
                            rhs=x32[:, ts(n, 512)],
                            start=(k == 0),
                            stop=(not has_bg) and k == KD - 1,
                        )
                if has_bg:
                    for n in range(NT):
                        nc.tensor.matmul(
                            g_ps[n][:],
                            lhsT=bg_sb[:1, :],
                            rhs=ones[:1, :],
                            start=False,
                            stop=True,
                        )
                for n in range(NT):
                    nc.vector.tensor_copy(gT[:, ts(n, 512)], g_ps[n][:])

                # transpose [E, T] -> [T-chunk, E] tiles
                for c in range(CT):
                    g_tr = ptr.tile([128, E], F32, name="g_tr")
                    nc.tensor.transpose(
                        g_tr[:], gT[:, ts(c, 128)], ident[:E, :E]
                    )
                    nc.vector.tensor_copy(gates[:, c, :], g_tr[:])

            # ---------------- top-2 + softmax -> w ---------------------------
            tk = ctx.enter_context(tc.tile_pool(name="topk", bufs=1))
            m1 = tk.tile([128, CT, 1], F32)
            m2 = tk.tile([128, CT, 1], F32)
            mask1 = tk.tile([128, CT, E], F32)
            mask2 = tk.tile([128, CT, E], F32)
            g2 = tk.tile([128, CT, E], F32)
            expd = tk.tile([128, CT, 1], F32)
            s1 = tk.tile([128, CT, 1], F32)
            s2 = tk.tile([128, CT, 1], F32)

            nc.vector.reduce_max(m1[:], gates[:], axis=AX.X)
            nc.vector.tensor_tensor(
                mask1[:], gates[:], m1[:].broadcast_to([128, CT, E]), op=ALU.is_equal
            )
            nc.vector.tensor_scalar(g2[:], mask1[:], NEG_BIG, None, op0=ALU.mult)
            nc.vector.tensor_add(g2[:], g2[:], gates[:])
            nc.vector.reduce_max(m2[:], g2[:], axis=AX.X)
            nc.vector.tensor_tensor(
                mask2[:], g2[:], m2[:].broadcast_to([128, CT, E]), op=ALU.is_equal
            )
            nc.vector.tensor_sub(expd[:], m2[:], m1[:])
            nc.scalar.activation(expd[:], expd[:], AF.Exp)
            nc.vector.tensor_scalar(s1[:], expd[:], 1.0, None, op0=ALU.add)
            nc.vector.reciprocal(s1[:], s1[:])
            nc.vector.tensor_mul(s2[:], expd[:], s1[:])
            nc.vector.tensor_tensor(
                mask1[:], mask1[:], s1[:].broadcast_to([128, CT, E]), op=ALU.mult
            )
            nc.vector.tensor_tensor(
                mask2[:], mask2[:], s2[:].broadcast_to([128, CT, E]), op=ALU.mult
            )
            nc.vector.tensor_add(w_sb[:], mask1[:], mask2[:])

            # optional b2 row replicated across partitions (via ones outer-prod)
            if has_b2:
                b2r_p = ctx.enter_context(tc.tile_pool(name="b2r", bufs=1))
                b2row_p = ctx.enter_context(tc.tile_pool(name="b2row", bufs=1))
                b2rep = b2r_p.tile([128, E, D], BF16)
                b2row = b2row_p.tile([1, E, D], F32)
                nc.sync.dma_start(b2row[:], b2_d.ap())
                with tc.tile_pool(name="pb2", bufs=2, space="PSUM") as pb2:
                    for e in range(E):
                        for nd in range(ND_):
                            b2ps = pb2.tile([128, 512], F32, name="b2ps")
                            nc.tensor.matmul(
                                b2ps[:],
                                lhsT=ones[:1, :128],
                                rhs=b2row[:, e, ts(nd, 512)],
                                start=True,
                                stop=True,
                            )
                            nc.vector.tensor_copy(b2rep[:, e, ts(nd, 512)], b2ps[:])

            # ---------------- expert loop ------------------------------------
            w1_p = ctx.enter_context(tc.tile_pool(name="w1", bufs=10))
            w2_p = ctx.enter_context(tc.tile_pool(name="w2", bufs=34))
            pmm1 = ctx.enter_context(tc.tile_pool(name="pmm1", bufs=4, space="PSUM"))
            pmm2 = ctx.enter_context(tc.tile_pool(name="pmm2", bufs=3, space="PSUM"))
            tmp_p = ctx.enter_context(tc.tile_pool(name="tmp", bufs=3))

            for e in range(E):
                hT = hT_p.tile([128, KH, TC], BF16, name="hT")

                # ---- mm1: hT = gelu(W1[e].T @ xT + b1[e]) ----
                for mb in range(MB):
                    w1t = []
                    for k in range(KD):
                        t = w1_p.tile([128, 1024], BF16, name="w1t")
                        nc.sync.dma_start(
                            t[:], W1_d.ap()[e, ts(k, 128), ts(mb, 1024)]
                        )
                        w1t.append(t)
                    for mi in range(8):
                        m = mb * 8 + mi
                        for n in range(NT):
                            ps = pmm1.tile([128, 512], F32, name="ps1")
                            for k in range(KD):
                                nc.tensor.matmul(
                                    ps[:],
                                    lhsT=w1t[k][:, ts(mi, 128)],
                                    rhs=xbf[:, k, ts(n, 512)],
                                    start=(k == 0),
                                    stop=(k == KD - 1),
                                )
                            bias = b1_sb[:, e, m : m + 1] if has_b1 else 0.0
                            nc.scalar.activation(
                                hT[:, m, ts(n, 512)], ps[:], AF.Gelu, bias=bias
                            )

                # ---- mm2: out += w[:, e] * (hT.T @ W2[e]) ----
                for nd in range(ND_):
                    w2t = []
                    for k in range(KH):
                        t = w2_p.tile([128, 512], BF16, name="w2t")
                        nc.sync.dma_start(
                            t[:], W2_d.ap()[e, ts(k, 128), ts(nd, 512)]
                        )
                        w2t.append(t)
                    for c in range(CT):
                        ps = pmm2.tile([128, 512], F32, name="ps2")
                        for k in range(KH):
                            nc.tensor.matmul(
                                ps[:],
                                lhsT=hT[:, k, ts(c, 128)],
                                rhs=w2t[k][:],
                                start=(k == 0),
                                stop=(k == KH - 1),
                            )
                        wv = w_sb[:, c, e : e + 1]
                        dst = out_sb[:, c, ts(nd, 512)]
                        if e == 0:
                            nc.vector.tensor_scalar(dst, ps[:], wv, None, op0=ALU.mult)
                        else:
                            tmp = tmp_p.tile([128, 512], F32, name="acc_tmp")
                            nc.vector.tensor_scalar(
                                tmp[:], ps[:], wv, None, op0=ALU.mult
                            )
                            nc.vector.tensor_add(dst, dst, tmp[:])

            # ---- optional b2 term: out += w @ b2 ----
            if has_b2:
                for c in range(CT):
                    for nd in range(ND_):
                        dst = out_sb[:, c, ts(nd, 512)]
                        for e in range(E):
                            tmp = tmp_p.tile([128, 512], F32, name="acc_tmp")
                            nc.vector.tensor_scalar(
                                tmp[:],
                                b2rep[:, e, ts(nd, 512)],
                                w_sb[:, c, e : e + 1],
                                None,
                                op0=ALU.mult,
                            )
                            nc.vector.tensor_add(dst, dst, tmp[:])

            # ---- store ----
            for c in range(CT):
                nc.sync.dma_start(out_d.ap()[ts(c, 128), :], out_sb[:, c, :])

    nc.compile()
    return nc


def _capacity_ok(xf, Wg, bg):
    """Host-side check only: if any (core, expert) routed count could exceed
    the compiled capacity C, fall back to the dense program (all math still
    happens on device either way)."""
    g = xf @ Wg + bg
    top2 = np.argpartition(-g, 2, axis=1)[:, :2]
    for c in range(N_CORES):
        t = top2[c * TC : (c + 1) * TC]
        for e in range(E):
            if (t == e).sum() > C - 8:
                return False
    return True


def kernel(x, Wg, bg, W1, b1, W2, b2):
    global LAST_EXEC_NS
    import os

    x = np.asarray(x, dtype=np.float32)
    Wg = np.asarray(Wg, dtype=np.float32)
    bg = np.asarray(bg, dtype=np.float32)
    W1 = np.asarray(W1, dtype=np.float32)
    b1 = np.asarray(b1, dtype=np.float32)
    W2 = np.asarray(W2, dtype=np.float32)
    b2 = np.asarray(b2, dtype=np.float32)

    B, S, _ = x.shape
    xf = x.reshape(-1, D)
    assert xf.shape[0] == N_CORES * TC

    has_bg = bool(np.any(bg))
    has_b1 = bool(np.any(b1))
    has_b2 = bool(np.any(b2))
    routed = _capacity_ok(xf, Wg, bg)
    key = (routed, has_bg, has_b1, has_b2)
    if key not in _PROGRAMS:
        build = _build_routed_program if routed else _build_dense_program
        _PROGRAMS[key] = build(has_bg, has_b1, has_b2)
    nc = _PROGRAMS[key]

    W1bf = W1.astype(ml_dtypes.bfloat16)
    W2bf = W2.astype(ml_dtypes.bfloat16)

    in_maps = []
    for c in range(N_CORES):
        shard = xf[c * TC : (c + 1) * TC]
        m = {
            "xT": np.ascontiguousarray(shard.T),
            "Wg": Wg,
            "W1": W1bf,
            "W2": W2bf,
        }
        if routed:
            m["xrows"] = shard.astype(ml_dtypes.bfloat16)
        if has_bg:
            m["bg"] = bg.reshape(1, E)
        if has_b1:
            m["b1c"] = np.ascontiguousarray(
                b1.reshape(E, H // 128, 128).transpose(2, 0, 1)
            )
        if has_b2:
            m["b2"] = b2.reshape(1, E, D)
        in_maps.append(m)

    trace = bool(os.environ.get("KERNEL_TRACE"))
    res = run_bass_kernel_spmd(
        nc, in_maps, core_ids=list(range(N_CORES)), trace=trace
    )
    LAST_EXEC_NS = res.exec_time_ns

    out = np.concatenate([res.results[c]["out"] for c in range(N_CORES)], axis=0)
    return out.reshape(B, S, D)
